# revision 18
# baseline (speedup 1.0000x reference)
"""AdaptiveRNNCell (ACT-halting GRU) Trainium2 kernel, 8-core data-parallel.

B=8192 batch sharded 1024/core; GRU weights replicated. All per-step state is
kept transposed [H-on-partitions, batch-on-free] so the recurrent matmul
h @ W_hh^T needs no per-step transposes. Weights are shipped pre-transposed /
tile-blocked in bf16 (host-side input marshaling), so the device does only
the input projection and the recurrent steps. Halting is per-sample; steps
1..19 are wrapped in runtime If(any_still_running) so the kernel stops
computing once every sample has halted (the torch module breaks early; with
halt bias 1.0 nearly everything halts after ~2 steps). Final scalar
statistics (means / quantiles / curve padding) are reduced on host from tiny
per-core vectors.
"""

import sys

for _p in ("/root/.axon_site/_ro/trn_rl_repo", "/opt/trn_rl_repo"):
    if _p not in sys.path:
        sys.path.append(_p)

import ml_dtypes
import numpy as np

import concourse.bass as bass
import concourse.mybir as mybir
import concourse.tile as tile
from concourse.masks import make_identity

N_CORES = 8
B_FULL, IN, H = 8192, 1024, 2048
B = B_FULL // N_CORES  # 1024 per core
G3 = 3 * H  # 6144
KT = H // 128  # 16 h tiles
GT = G3 // 128  # 48 gate tiles
IT = IN // 128  # 8 input tiles
CH = 512  # matmul moving chunk
NCH = B // CH  # 2
T = 20
THRESH = np.float32(1.0 - 0.01)
TIME_PENALTY = np.float32(0.001)

F32 = mybir.dt.float32
BF16 = mybir.dt.bfloat16
I32 = mybir.dt.int32
AF = mybir.ActivationFunctionType
ALU = mybir.AluOpType
AX = mybir.AxisListType


# ---------------------------------------------------------------- shims ----
def _patch_tile_drain():
    """walrus here rejects >1 sem wait on CTRL instructions: split the tile
    kernel-tail drain's waits across single-wait NOPs."""
    if getattr(tile.TileContext, "_drain_patched", False):
        return
    from concourse.vector_clock import ScopedClock

    def _patched(self, tick_clock, wait_clock):
        nc = self.nc
        drain_inst = nc.sync.drain()
        wait_clock.add_sem_waits(
            drain_inst.ins, ScopedClock({None: tick_clock.global_clock})
        )
        waits = list(drain_inst.ins.sync_info.on_wait)
        if len(waits) > 1:
            drain_inst.ins.sync_info = mybir.SyncInfo(
                on_wait=waits[:1], on_update=[]
            )
            for w in waits[1:]:
                nop = nc.sync.nop(nofuse=True)
                nop.ins.sync_info = mybir.SyncInfo(on_wait=[w], on_update=[])
        nc.all_engine_barrier()
        assert self.sems is not None
        popped = nc._tile_sem_poison_stack.pop()
        assert popped is self._sem_poison
        nc.clear_and_free_semaphores(list(self.sems.allocated().values()))
        nc.all_engine_barrier()

    tile.TileContext._drain_and_barrier = _patched
    tile.TileContext._drain_patched = True


def _split_excess_waits(nc, limit=1, max_upd=63):
    """walrus here caps sem waits per instruction and only supports small
    sem increments on compute instructions. Hoist excess waits onto
    same-engine NOPs and oversized sem-add updates onto EventSemaphore
    instructions emitted right after the owner."""
    n_fixed = 0
    for fn in nc.m.functions:
        for blk in fn.blocks:
            changed = False
            new_list = []
            for inst in blk.instructions:
                si = inst.sync_info
                waits = list(si.on_wait) if si is not None else []
                upds = list(si.on_update) if si is not None else []
                big_upds = [
                    u for u in upds
                    if getattr(u, "update_mode", "") == "sem-add-imm"
                    and getattr(u, "update_value", 0) > 1
                    and inst.opcode not in ("EventSemaphore", "ISA")
                ]
                if len(waits) > limit or big_upds:
                    hoist, keep = waits[:-limit], waits[-limit:]
                    for w in hoist:
                        n_fixed += 1
                        nop = mybir.InstNoOp(
                            name=f"waitsplit-{n_fixed}-{inst.name}", ins=[], outs=[]
                        )
                        nop.engine = inst.engine
                        nop.sync_info = mybir.SyncInfo(on_wait=[w], on_update=[])
                        new_list.append(nop)
                    keep_upds = [u for u in upds if u not in big_upds]
                    tail = []
                    for u in big_upds:
                        left = u.update_value - 1
                        ku = mybir.SyncUpdate(
                            ant_name=u.ant_name, id=u.id,
                            sync_type=u.sync_type,
                            update_mode="sem-inc", update_value=1)
                        keep_upds.append(ku)
                        while left > 0:
                            n_fixed += 1
                            cu = mybir.SyncUpdate(
                                ant_name=u.ant_name, id=u.id,
                                sync_type=u.sync_type,
                                update_mode="sem-add-imm",
                                update_value=min(left, max_upd))
                            ev = mybir.InstEventSemaphore(
                                name=f"updsplit-{n_fixed}-{inst.name}",
                                ins=[], outs=[])
                            ev.engine = inst.engine
                            ev.sync_info = mybir.SyncInfo(on_wait=[],
                                                          on_update=[cu])
                            tail.append(ev)
                            left -= max_upd
                    inst.sync_info = mybir.SyncInfo(
                        on_wait=keep, on_update=keep_upds
                    )
                    new_list.append(inst)
                    new_list.extend(tail)
                    changed = True
                else:
                    new_list.append(inst)
            if changed:
                blk.instructions = new_list
    return n_fixed


# ------------------------------------------------------------- builder ----
def _build(n_steps=T):
    _patch_tile_drain()
    nc = bass.Bass("TRN2", target_bir_lowering=False, debug=False,
                   num_devices=N_CORES)

    # host-marshaled inputs (pre-transposed / tile-blocked / pre-cast)
    xt_e = nc.dram_tensor("x_t", [128, IT * B], BF16, kind="ExternalInput")
    wih_e = nc.dram_tensor("w_ih_t", [GT, 128, IT * 128], BF16,
                           kind="ExternalInput")
    whh_e = nc.dram_tensor("w_hh_t", [GT, 128, KT * 128], BF16,
                           kind="ExternalInput")
    bih_e = nc.dram_tensor("bih_p", [128, GT], F32, kind="ExternalInput")
    bhh_e = nc.dram_tensor("bhh_p", [128, GT], F32, kind="ExternalInput")
    wf_e = nc.dram_tensor("wf_p", [128, GT], F32, kind="ExternalInput")
    hw_e = nc.dram_tensor("hw_p", [128, KT], BF16, kind="ExternalInput")
    hb_e = nc.dram_tensor("hb_p", [1, 1], F32, kind="ExternalInput")

    acc_e = nc.dram_tensor("acc_t", [H, B], F32, kind="ExternalOutput")
    stats_e = nc.dram_tensor("stats", [4, B], F32, kind="ExternalOutput")
    curve_e = nc.dram_tensor("curve", [1, 32], F32, kind="ExternalOutput")

    with tile.TileContext(nc) as tc:
        _body(nc, tc, n_steps, xt_e, wih_e, whh_e, bih_e, bhh_e, wf_e,
              hw_e, hb_e, acc_e, stats_e, curve_e)

    nfix = _split_excess_waits(nc, limit=1)
    return nc, nfix


def _body(nc, tc, n_steps, xt_e, wih_e, whh_e, bih_e, bhh_e, wf_e,
          hw_e, hb_e, acc_e, stats_e, curve_e):
    from contextlib import ExitStack

    with ExitStack() as st:
        cpool = st.enter_context(tc.tile_pool(name="const", bufs=1))
        wpool = st.enter_context(tc.tile_pool(name="wstream", bufs=6))
        xwpool = st.enter_context(tc.tile_pool(name="xwstream", bufs=4))
        gpool = st.enter_context(tc.tile_pool(name="gates", bufs=2))
        pspool = st.enter_context(tc.tile_pool(name="ps", bufs=6, space="PSUM"))
        pppool = st.enter_context(tc.tile_pool(name="pp", bufs=2, space="PSUM"))
        dpool = st.enter_context(tc.tile_pool(name="dram", bufs=1, space="DRAM"))

        # ---- constants -----------------------------------------------
        ident_b = cpool.tile([128, 128], BF16, tag="identb")
        make_identity(nc, ident_b[:])
        ones_col = cpool.tile([1, 128], F32, tag="ones")
        nc.gpsimd.memset(ones_col[:], 1.0)

        halt_w_sb = cpool.tile([128, KT], BF16, tag="haltw")
        nc.sync.dma_start(halt_w_sb[:], hw_e.ap())
        hb_sb = cpool.tile([1, 1], F32, tag="haltb")
        nc.sync.dma_start(hb_sb[:], hb_e.ap())

        bih_sb = cpool.tile([128, GT], F32, tag="bih")
        bhh_sb = cpool.tile([128, GT], F32, tag="bhh")
        wf_sb = cpool.tile([128, GT], F32, tag="wf")
        nc.sync.dma_start(bih_sb[:], bih_e.ap())
        nc.sync.dma_start(bhh_sb[:], bhh_e.ap())
        nc.sync.dma_start(wf_sb[:], wf_e.ap())
        bias_sum = cpool.tile([128, GT], F32, tag="bsum")  # b_ih + b_hh
        nc.vector.tensor_tensor(bias_sum[:], bih_sb[:], bhh_sb[:], ALU.add)
        bias0_sum = cpool.tile([128, GT], F32, tag="b0sum")  # + w_flag (step 0)
        nc.vector.tensor_tensor(bias0_sum[:], bias_sum[:], wf_sb[:], ALU.add)
        bias0_n = cpool.tile([128, GT], F32, tag="b0n")  # b_ih + w_flag (step 0)
        nc.vector.tensor_tensor(bias0_n[:], bih_sb[:], wf_sb[:], ALU.add)

        # ---- persistent per-sample rows ------------------------------
        def row(tag, init=0.0):
            t = cpool.tile([1, B], F32, tag=tag, name=f"row_{tag}")
            nc.gpsimd.memset(t[:], init)
            return t

        still = row("still", 1.0)
        acc_p = row("accp")
        steps_r = row("steps")
        rem_v = row("remv")
        nat_r = row("nat")
        forc_r = row("forc")
        p_sb = row("prow")
        new_acc = row("newacc")
        one_m = row("onem")
        natf = row("natf")
        p_adj = row("padj")
        rem = row("rem")
        sel = row("sel")

        curve_sb = cpool.tile([1, 32], F32, tag="curve")
        nc.gpsimd.memset(curve_sb[:], 0.0)
        count_sb = cpool.tile([1, 1], F32, tag="count")
        nc.gpsimd.memset(count_sb[:], 1.0)

        # ---- DRAM scratch --------------------------------------------
        xw_ts = [dpool.tile([128, B], BF16, tag=f"xw{j}", name=f"xwts{j}")
                 for j in range(GT)]
        acc_ts = [[dpool.tile([128, CH], F32, tag=f"acc{i}_{c}",
                              name=f"accts{i}_{c}")
                   for c in range(NCH)] for i in range(KT)]

        # ---- input projection: xw_ts[jt] = (x @ W_ih[:, :IN].T)^T ----
        with tc.tile_pool(name="xtp", bufs=1) as xtpool:
            xT = xtpool.tile([128, IT * B], BF16, tag="xT")
            nc.sync.dma_start(xT[:], xt_e.ap())
            proj_order = [g * KT + i for i in range(KT) for g in range(3)]
            for jt in proj_order:
                wT = wpool.tile([128, IT * 128], BF16, tag="wblk")
                nc.sync.dma_start(wT[:], wih_e.ap()[jt])
                xst = xwpool.tile([128, B], BF16, tag="xwst")
                for c in range(NCH):
                    px = pspool.tile([128, CH], F32, tag="ps")
                    for it in range(IT):
                        nc.tensor.matmul(
                            px[:],
                            wT[:, it * 128:(it + 1) * 128],
                            xT[:, it * B + c * CH: it * B + c * CH + CH],
                            start=(it == 0), stop=(it == IT - 1),
                        )
                    nc.vector.tensor_copy(xst[:, c * CH:(c + 1) * CH], px[:])
                nc.sync.dma_start(xw_ts[jt][:], xst[:])

        # ---- h ping-pong ---------------------------------------------
        hpool = st.enter_context(tc.tile_pool(name="hbuf", bufs=1))
        h_a = hpool.tile([128, KT * B], BF16, tag="ha")
        h_b = hpool.tile([128, KT * B], BF16, tag="hb")
        hbufs = [h_a, h_b]

        # ---- halting tail (shared by all steps) ----------------------
        def acc_pass(t, h_nxt):
            # acc_state accumulation: acc += p_adj * h_nxt  (HWDGE RMW)
            for c in range(NCH):
                pP = pspool.tile([128, CH], F32, tag="ps")
                nc.tensor.matmul(pP[:], ones_col[:],
                                 p_adj[0:1, c * CH:(c + 1) * CH],
                                 start=True, stop=True)
                for i in range(KT):
                    tmp = gpool.tile([128, CH], F32, tag="acctmp")
                    nc.vector.tensor_tensor(
                        tmp[:], pP[:],
                        h_nxt[:, i * B + c * CH: i * B + c * CH + CH],
                        ALU.mult)
                    if t == 0:
                        nc.sync.dma_start(acc_ts[i][c][:], tmp[:])
                    else:
                        a_in = gpool.tile([128, CH], F32, tag="accin", bufs=4)
                        nc.sync.dma_start(a_in[:], acc_ts[i][c][:])
                        a_new = gpool.tile([128, CH], F32, tag="accnew",
                                           bufs=2)
                        eng = nc.vector if i % 2 == 0 else nc.gpsimd
                        eng.tensor_tensor(a_new[:], a_in[:], tmp[:], ALU.add)
                        nc.sync.dma_start(acc_ts[i][c][:], a_new[:])

        def halt_tail(t, h_nxt, ps_p, defer_acc=False):
            for c in range(NCH):
                nc.scalar.activation(p_sb[0:1, c * CH:(c + 1) * CH],
                                     ps_p[c][:], AF.Sigmoid, bias=hb_sb[:])
            nc.vector.tensor_tensor(new_acc[:], acc_p[:], p_sb[:], ALU.add)
            nc.gpsimd.tensor_scalar(one_m[:], acc_p[:], -1.0, 1.0,
                                    ALU.mult, ALU.add)
            nc.vector.tensor_scalar(natf[:], new_acc[:], float(THRESH),
                                    None, ALU.is_ge)
            nc.vector.tensor_tensor(natf[:], natf[:], still[:], ALU.mult)
            if t == T - 1:
                # forced halt of everyone still running
                nc.vector.tensor_tensor(p_adj[:], still[:], one_m[:], ALU.mult)
                nc.vector.tensor_copy(rem[:], p_adj[:])
                nc.vector.tensor_tensor(sel[:], still[:], natf[:],
                                        ALU.subtract)
                nc.vector.tensor_tensor(forc_r[:], forc_r[:], sel[:], ALU.add)
            else:
                # where(natf, one_m, p) == p + natf * (one_m - p)
                nc.vector.tensor_tensor(sel[:], one_m[:], p_sb[:],
                                        ALU.subtract)
                nc.vector.tensor_tensor(sel[:], natf[:], sel[:], ALU.mult)
                nc.vector.tensor_tensor(sel[:], sel[:], p_sb[:], ALU.add)
                nc.vector.tensor_tensor(p_adj[:], sel[:], still[:], ALU.mult)
                nc.vector.tensor_tensor(rem[:], natf[:], one_m[:], ALU.mult)
            nc.vector.scalar_tensor_tensor(
                acc_p[:], acc_p[:], 0.0, p_adj[:], ALU.add, ALU.add,
                accum_out=curve_sb[0:1, t:t + 1],
            )
            nc.gpsimd.tensor_tensor(steps_r[:], steps_r[:], still[:], ALU.add)
            nc.gpsimd.tensor_tensor(rem_v[:], rem_v[:], rem[:], ALU.add)
            nc.gpsimd.tensor_tensor(nat_r[:], nat_r[:], natf[:], ALU.add)
            if t == T - 1:
                nc.gpsimd.memset(still[:], 0.0)
            else:
                nc.vector.tensor_tensor(still[:], still[:], natf[:],
                                        ALU.subtract)
                nc.vector.tensor_reduce(count_sb[:], still[:], AX.X, ALU.add)
            if not defer_acc:
                acc_pass(t, h_nxt)

        # ---- step 0 (h=0: gates come from xW + biases only) ----------
        h_nxt = hbufs[1]
        ps_p0 = [pppool.tile([1, CH], F32, tag="pp", name=f"psp0_{c}")
                 for c in range(NCH)]
        for i in range(KT):
            xw_r = xwpool.tile([128, B], BF16, tag="xw")
            xw_z = xwpool.tile([128, B], BF16, tag="xw")
            xw_n = xwpool.tile([128, B], BF16, tag="xw")
            nc.sync.dma_start(xw_r[:], xw_ts[i][:])
            nc.sync.dma_start(xw_z[:], xw_ts[KT + i][:])
            nc.sync.dma_start(xw_n[:], xw_ts[2 * KT + i][:])
            hs0 = slice(i * B, (i + 1) * B)
            r0 = gpool.tile([128, B], BF16, tag="r")
            nc.scalar.activation(r0[:], xw_r[:], AF.Sigmoid,
                                 bias=bias0_sum[:, i:i + 1])
            z0 = gpool.tile([128, B], BF16, tag="z")
            nc.scalar.activation(z0[:], xw_z[:], AF.Sigmoid,
                                 bias=bias0_sum[:, KT + i:KT + i + 1])
            u0 = gpool.tile([128, B], BF16, tag="u")
            nc.vector.scalar_tensor_tensor(
                u0[:], r0[:], bhh_sb[:, 2 * KT + i:2 * KT + i + 1],
                xw_n[:], ALU.mult, ALU.add)
            n0 = gpool.tile([128, B], BF16, tag="n")
            nc.scalar.activation(n0[:], u0[:], AF.Tanh,
                                 bias=bias0_n[:, 2 * KT + i:2 * KT + i + 1])
            e0 = gpool.tile([128, B], BF16, tag="e")
            nc.vector.tensor_tensor(e0[:], z0[:], n0[:], ALU.mult)
            nc.vector.tensor_tensor(h_nxt[:, hs0], n0[:], e0[:], ALU.subtract)
            for c in range(NCH):
                nc.tensor.matmul(
                    ps_p0[c][:], halt_w_sb[:, i:i + 1],
                    h_nxt[:, i * B + c * CH: i * B + (c + 1) * CH],
                    start=(i == 0), stop=(i == KT - 1))
        halt_tail(0, h_nxt, ps_p0)

        # ---- steps 1..n_steps-1 with early exit ----------------------
        def step_body(t):
            h_cur = hbufs[t % 2]
            h_nx = hbufs[(t + 1) % 2]
            ps_p_t = [pppool.tile([1, CH], F32, tag="pp", name=f"pspt{t}_{c}")
                      for c in range(NCH)]
            for i in range(KT):
                jr, jz, jn = i, KT + i, 2 * KT + i
                w_r = wpool.tile([128, KT * 128], BF16, tag="wblk")
                w_z = wpool.tile([128, KT * 128], BF16, tag="wblk")
                w_n = wpool.tile([128, KT * 128], BF16, tag="wblk")
                nc.sync.dma_start(w_r[:], whh_e.ap()[jr])
                nc.sync.dma_start(w_z[:], whh_e.ap()[jz])
                nc.sync.dma_start(w_n[:], whh_e.ap()[jn])
                xw_r = xwpool.tile([128, B], BF16, tag="xw")
                xw_z = xwpool.tile([128, B], BF16, tag="xw")
                xw_n = xwpool.tile([128, B], BF16, tag="xw")
                nc.sync.dma_start(xw_r[:], xw_ts[jr][:])
                nc.sync.dma_start(xw_z[:], xw_ts[jz][:])
                nc.sync.dma_start(xw_n[:], xw_ts[jn][:])
                for c in range(NCH):
                    sl = slice(c * CH, (c + 1) * CH)
                    hsl = slice(i * B + c * CH, i * B + (c + 1) * CH)
                    ps_r = pspool.tile([128, CH], F32, tag="ps")
                    ps_z = pspool.tile([128, CH], F32, tag="ps")
                    ps_n = pspool.tile([128, CH], F32, tag="ps")
                    for kt in range(KT):
                        rh = h_cur[:, kt * B + c * CH: kt * B + (c + 1) * CH]
                        nc.tensor.matmul(
                            ps_r[:], w_r[:, kt * 128:(kt + 1) * 128], rh,
                            start=(kt == 0), stop=False)
                        nc.tensor.matmul(
                            ps_z[:], w_z[:, kt * 128:(kt + 1) * 128], rh,
                            start=(kt == 0), stop=False)
                        nc.tensor.matmul(
                            ps_n[:], w_n[:, kt * 128:(kt + 1) * 128], rh,
                            start=(kt == 0), stop=(kt == KT - 1))
                    nc.tensor.matmul(ps_r[:], ident_b[:], xw_r[:, sl],
                                     start=False, stop=True)
                    nc.tensor.matmul(ps_z[:], ident_b[:], xw_z[:, sl],
                                     start=False, stop=True)
                    r_t = gpool.tile([128, CH], BF16, tag="r")
                    nc.scalar.activation(r_t[:], ps_r[:], AF.Sigmoid,
                                         bias=bias_sum[:, jr:jr + 1])
                    z_t = gpool.tile([128, CH], BF16, tag="z")
                    nc.scalar.activation(z_t[:], ps_z[:], AF.Sigmoid,
                                         bias=bias_sum[:, jz:jz + 1])
                    u_t = gpool.tile([128, CH], BF16, tag="u")
                    nc.vector.scalar_tensor_tensor(
                        u_t[:], ps_n[:], bhh_sb[:, jn:jn + 1], r_t[:],
                        ALU.add, ALU.mult)
                    v_t = gpool.tile([128, CH], BF16, tag="v")
                    nc.vector.tensor_tensor(v_t[:], u_t[:], xw_n[:, sl],
                                            ALU.add)
                    n_t = gpool.tile([128, CH], BF16, tag="n")
                    nc.scalar.activation(n_t[:], v_t[:], AF.Tanh,
                                         bias=bih_sb[:, jn:jn + 1])
                    d_t = gpool.tile([128, CH], BF16, tag="d")
                    nc.vector.tensor_tensor(d_t[:], h_cur[:, hsl], n_t[:],
                                            ALU.subtract)
                    e_t = gpool.tile([128, CH], BF16, tag="e")
                    nc.vector.tensor_tensor(e_t[:], z_t[:], d_t[:], ALU.mult)
                    nc.vector.tensor_tensor(h_nx[:, hsl], n_t[:], e_t[:],
                                            ALU.add)
                    nc.tensor.matmul(ps_p_t[c][:], halt_w_sb[:, i:i + 1],
                                     h_nx[:, hsl],
                                     start=(i == 0), stop=(i == KT - 1))
            halt_tail(t, h_nx, ps_p_t)

        def load_count():
            return nc.values_load(
                count_sb[:].bitcast(I32), min_val=0, max_val=2 ** 30,
                skip_runtime_bounds_check=True)

        if n_steps > 1:
            c1 = load_count()
            with tc.If(c1 > 0):
                step_body(1)
        if n_steps > 2:
            c2 = load_count()
            with tc.If(c2 > 0):
                for t in range(2, n_steps):
                    ct = load_count()
                    with tc.If(ct > 0):
                        step_body(t)

        # ---- final outputs -------------------------------------------
        for i in range(KT):
            for c in range(NCH):
                nc.sync.dma_start(
                    acc_e.ap()[i * 128:(i + 1) * 128, c * CH:(c + 1) * CH],
                    acc_ts[i][c][:])
        nc.sync.dma_start(stats_e.ap()[0:1, :], steps_r[:])
        nc.sync.dma_start(stats_e.ap()[1:2, :], rem_v[:])
        nc.sync.dma_start(stats_e.ap()[2:3, :], nat_r[:])
        nc.sync.dma_start(stats_e.ap()[3:4, :], forc_r[:])
        nc.sync.dma_start(curve_e.ap()[:, :], curve_sb[:])


# --------------------------------------------------------------- runner ----
_CACHE = {}


def _get_nc(n_steps=T):
    if n_steps not in _CACHE:
        _CACHE[n_steps] = _build(n_steps)[0]
    return _CACHE[n_steps]


def _marshal(inputs):
    """Host-side input marshaling: shard x, pre-transpose / tile-block /
    bf16-cast the replicated weights into the layouts the device consumes."""
    bf = ml_dtypes.bfloat16
    x = np.asarray(inputs["input_tensor"], dtype=np.float32)
    w_ih = np.asarray(inputs["weight_ih"], dtype=np.float32)
    w_hh = np.asarray(inputs["weight_hh"], dtype=np.float32)
    b_ih = np.asarray(inputs["bias_ih"], dtype=np.float32)
    b_hh = np.asarray(inputs["bias_hh"], dtype=np.float32)
    halt_w = np.asarray(inputs["halt_w"], dtype=np.float32)
    halt_b = np.asarray(inputs["halt_b"], dtype=np.float32)

    # w_ih_t[jt, p, it*128+j'] = w_ih[jt*128+j', it*128+p]
    wih_blk = w_ih[:, :IN].reshape(GT, 128, IT, 128)  # [jt, j', it, p]
    wih_blk = np.ascontiguousarray(
        wih_blk.transpose(0, 3, 2, 1).reshape(GT, 128, IT * 128)
    ).astype(bf)

    # w_hh_t[jt, p, kt*128+j'] = w_hh[jt*128+j', kt*128+p]
    whh_blk = w_hh.reshape(GT, 128, KT, 128)  # [jt, j', kt, p]
    whh_blk = np.ascontiguousarray(
        whh_blk.transpose(0, 3, 2, 1).reshape(GT, 128, KT * 128)
    ).astype(bf)

    bih_p = np.ascontiguousarray(b_ih.reshape(GT, 128).T)
    bhh_p = np.ascontiguousarray(b_hh.reshape(GT, 128).T)
    wf_p = np.ascontiguousarray(w_ih[:, IN].reshape(GT, 128).T)
    hw_p = np.ascontiguousarray(halt_w.reshape(KT, 128).T.astype(bf))
    hb_p = np.ascontiguousarray(halt_b.reshape(1, 1))

    base = {
        "w_ih_t": wih_blk, "w_hh_t": whh_blk,
        "bih_p": bih_p, "bhh_p": bhh_p, "wf_p": wf_p,
        "hw_p": hw_p, "hb_p": hb_p,
    }
    in_maps = []
    for ci in range(N_CORES):
        xs = x[ci * B:(ci + 1) * B]  # [B, IN]
        # x_t[p, it*B + b] = x[b, it*128+p]
        xt = np.ascontiguousarray(
            xs.reshape(B, IT, 128).transpose(2, 1, 0).reshape(128, IT * B)
        ).astype(bf)
        in_maps.append(dict(base, x_t=xt))
    return in_maps


def run_device(inputs, n_steps=T, trace=False):
    from concourse.bass_utils import run_bass_kernel_spmd

    nc = _get_nc(n_steps)
    in_maps = _marshal(inputs)
    return run_bass_kernel_spmd(nc, in_maps, core_ids=list(range(N_CORES)),
                                trace=trace)


def combine(results):
    """Host-side unshard + tiny final statistics."""
    accs, steps_l, rem_l, curves, tmaxes = [], [], [], [], []
    nat_s = forc_s = 0.0
    for ci in range(N_CORES):
        r = results[ci]
        accs.append(np.ascontiguousarray(r["acc_t"].T))
        stats = r["stats"]
        steps_l.append(stats[0])
        rem_l.append(stats[1])
        nat_s += float(stats[2].sum(dtype=np.float64))
        forc_s += float(stats[3].sum(dtype=np.float64))
        curves.append(r["curve"][0, :T].copy())
        tmaxes.append(int(stats[0].max()))

    acc_state = np.concatenate(accs, axis=0)
    steps = np.concatenate(steps_l)
    rem_v = np.concatenate(rem_l)

    # pad each core's curve past its last executed step with its final value
    curve = np.zeros(T, dtype=np.float64)
    for ci in range(N_CORES):
        c = curves[ci].astype(np.float64)
        tm = max(tmaxes[ci], 1)
        c[tm:] = c[tm - 1]
        curve += c
    curve = (curve / B_FULL).astype(np.float32)

    ponder = (steps + rem_v) * TIME_PENALTY
    final_ponder = np.float32(ponder.mean(dtype=np.float64))
    remainder_mean = np.float32(rem_v.mean(dtype=np.float64))
    remainder_std = np.float32(rem_v.std(dtype=np.float64))
    natural_ratio = np.float32(nat_s / B_FULL)
    forced_ratio = np.float32(forc_s / B_FULL)
    p50 = np.float32(np.quantile(steps.astype(np.float64), 0.5))
    p90 = np.float32(np.quantile(steps.astype(np.float64), 0.9))
    return (acc_state, final_ponder, steps, remainder_mean, remainder_std,
            natural_ratio, forced_ratio, p50, p90, curve)


def kernel(**inputs):
    res = run_device(inputs, n_steps=T, trace=False)
    return combine(res.results)


if __name__ == "__main__":
    import os
    import time

    t0 = time.time()
    nc, nfix = _build(int(os.environ.get("NSTEPS", T)))
    print(f"built ok in {time.time() - t0:.1f}s, waitsplit fixes: {nfix}")


# revision 19
# speedup vs baseline: 5.5409x; 5.5409x over previous
"""AdaptiveRNNCell (ACT-halting GRU) Trainium2 kernel, 8-core data-parallel.

B=8192 batch sharded 1024/core; GRU weights replicated. All per-step state is
kept transposed [H-on-partitions, batch-on-free] so the recurrent matmul
h @ W_hh^T needs no per-step transposes. Weights are shipped pre-transposed /
tile-blocked in bf16 (host-side input marshaling), so the device does only
the input projection and the recurrent steps. Halting is per-sample; steps
1..19 are wrapped in runtime If(any_still_running) so the kernel stops
computing once every sample has halted (the torch module breaks early; with
halt bias 1.0 nearly everything halts after ~2 steps). Final scalar
statistics (means / quantiles / curve padding) are reduced on host from tiny
per-core vectors.
"""

import sys

for _p in ("/root/.axon_site/_ro/trn_rl_repo", "/opt/trn_rl_repo"):
    if _p not in sys.path:
        sys.path.append(_p)

import ml_dtypes
import numpy as np

import concourse.bass as bass
import concourse.mybir as mybir
import concourse.tile as tile
from concourse.masks import make_identity

N_CORES = 8
B_FULL, IN, H = 8192, 1024, 2048
B = B_FULL // N_CORES  # 1024 per core
G3 = 3 * H  # 6144
KT = H // 128  # 16 h tiles
GT = G3 // 128  # 48 gate tiles
IT = IN // 128  # 8 input tiles
CH = 512  # matmul moving chunk
NCH = B // CH  # 2
T = 20
THRESH = np.float32(1.0 - 0.01)
TIME_PENALTY = np.float32(0.001)

F32 = mybir.dt.float32
BF16 = mybir.dt.bfloat16
I32 = mybir.dt.int32
AF = mybir.ActivationFunctionType
ALU = mybir.AluOpType
AX = mybir.AxisListType


# ---------------------------------------------------------------- shims ----
def _patch_tile_drain():
    """walrus here rejects >1 sem wait on CTRL instructions: split the tile
    kernel-tail drain's waits across single-wait NOPs."""
    if getattr(tile.TileContext, "_drain_patched", False):
        return
    from concourse.vector_clock import ScopedClock

    def _patched(self, tick_clock, wait_clock):
        nc = self.nc
        drain_inst = nc.sync.drain()
        wait_clock.add_sem_waits(
            drain_inst.ins, ScopedClock({None: tick_clock.global_clock})
        )
        waits = list(drain_inst.ins.sync_info.on_wait)
        if len(waits) > 1:
            drain_inst.ins.sync_info = mybir.SyncInfo(
                on_wait=waits[:1], on_update=[]
            )
            for w in waits[1:]:
                nop = nc.sync.nop(nofuse=True)
                nop.ins.sync_info = mybir.SyncInfo(on_wait=[w], on_update=[])
        nc.all_engine_barrier()
        assert self.sems is not None
        popped = nc._tile_sem_poison_stack.pop()
        assert popped is self._sem_poison
        nc.clear_and_free_semaphores(list(self.sems.allocated().values()))
        nc.all_engine_barrier()

    tile.TileContext._drain_and_barrier = _patched
    tile.TileContext._drain_patched = True


def _split_excess_waits(nc, limit=1, max_upd=63):
    """walrus here caps sem waits per instruction and only supports small
    sem increments on compute instructions. Hoist excess waits onto
    same-engine NOPs and oversized sem-add updates onto EventSemaphore
    instructions emitted right after the owner."""
    n_fixed = 0
    for fn in nc.m.functions:
        for blk in fn.blocks:
            changed = False
            new_list = []
            for inst in blk.instructions:
                si = inst.sync_info
                waits = list(si.on_wait) if si is not None else []
                upds = list(si.on_update) if si is not None else []
                big_upds = [
                    u for u in upds
                    if getattr(u, "update_mode", "") == "sem-add-imm"
                    and getattr(u, "update_value", 0) > 1
                    and inst.opcode not in ("EventSemaphore", "ISA",
                                            "DMACopy", "Drain", "NoOp")
                ]
                if len(waits) > limit or big_upds:
                    hoist, keep = waits[:-limit], waits[-limit:]
                    for w in hoist:
                        n_fixed += 1
                        nop = mybir.InstNoOp(
                            name=f"waitsplit-{n_fixed}-{inst.name}", ins=[], outs=[]
                        )
                        nop.engine = inst.engine
                        nop.sync_info = mybir.SyncInfo(on_wait=[w], on_update=[])
                        new_list.append(nop)
                    keep_upds = [u for u in upds if u not in big_upds]
                    tail = []
                    for u in big_upds:
                        left = u.update_value - 1
                        ku = mybir.SyncUpdate(
                            ant_name=u.ant_name, id=u.id,
                            sync_type=u.sync_type,
                            update_mode="sem-inc", update_value=1)
                        keep_upds.append(ku)
                        while left > 0:
                            n_fixed += 1
                            cu = mybir.SyncUpdate(
                                ant_name=u.ant_name, id=u.id,
                                sync_type=u.sync_type,
                                update_mode="sem-add-imm",
                                update_value=min(left, max_upd))
                            ev = mybir.InstEventSemaphore(
                                name=f"updsplit-{n_fixed}-{inst.name}",
                                ins=[], outs=[])
                            ev.engine = inst.engine
                            ev.sync_info = mybir.SyncInfo(on_wait=[],
                                                          on_update=[cu])
                            tail.append(ev)
                            left -= max_upd
                    inst.sync_info = mybir.SyncInfo(
                        on_wait=keep, on_update=keep_upds
                    )
                    new_list.append(inst)
                    new_list.extend(tail)
                    changed = True
                else:
                    new_list.append(inst)
            if changed:
                blk.instructions = new_list
    return n_fixed


# ------------------------------------------------------------- builder ----
def _build(n_steps=T):
    _patch_tile_drain()
    nc = bass.Bass("TRN2", target_bir_lowering=False, debug=False,
                   num_devices=N_CORES)

    # host-marshaled inputs (pre-transposed / tile-blocked / pre-cast)
    xt_e = nc.dram_tensor("x_t", [128, IT * B], BF16, kind="ExternalInput")
    wih_e = nc.dram_tensor("w_ih_t", [GT, 128, IT * 128], BF16,
                           kind="ExternalInput")
    whh_e = nc.dram_tensor("w_hh_t", [GT, 128, KT * 128], BF16,
                           kind="ExternalInput")
    bih_e = nc.dram_tensor("bih_p", [128, GT], F32, kind="ExternalInput")
    bhh_e = nc.dram_tensor("bhh_p", [128, GT], F32, kind="ExternalInput")
    wf_e = nc.dram_tensor("wf_p", [128, GT], F32, kind="ExternalInput")
    hw_e = nc.dram_tensor("hw_p", [128, KT], BF16, kind="ExternalInput")
    hb_e = nc.dram_tensor("hb_p", [1, 1], F32, kind="ExternalInput")

    acc_e = nc.dram_tensor("acc_t", [H, B], F32, kind="ExternalOutput")
    stats_e = nc.dram_tensor("stats", [4, B], F32, kind="ExternalOutput")
    curve_e = nc.dram_tensor("curve", [1, 32], F32, kind="ExternalOutput")

    with tile.TileContext(nc) as tc:
        _body(nc, tc, n_steps, xt_e, wih_e, whh_e, bih_e, bhh_e, wf_e,
              hw_e, hb_e, acc_e, stats_e, curve_e)

    nfix = _split_excess_waits(nc, limit=1)
    return nc, nfix


def _body(nc, tc, n_steps, xt_e, wih_e, whh_e, bih_e, bhh_e, wf_e,
          hw_e, hb_e, acc_e, stats_e, curve_e):
    from contextlib import ExitStack

    with ExitStack() as st:
        cpool = st.enter_context(tc.tile_pool(name="const", bufs=1))
        wpool = st.enter_context(tc.tile_pool(name="wstream", bufs=6))
        xwpool = st.enter_context(tc.tile_pool(name="xwstream", bufs=4))
        gpool = st.enter_context(tc.tile_pool(name="gates", bufs=2))
        pspool = st.enter_context(tc.tile_pool(name="ps", bufs=6, space="PSUM"))
        pppool = st.enter_context(tc.tile_pool(name="pp", bufs=2, space="PSUM"))
        dpool = st.enter_context(tc.tile_pool(name="dram", bufs=1, space="DRAM"))

        # ---- constants -----------------------------------------------
        ident_b = cpool.tile([128, 128], BF16, tag="identb")
        make_identity(nc, ident_b[:])
        ones_col = cpool.tile([1, 128], F32, tag="ones")
        nc.gpsimd.memset(ones_col[:], 1.0)

        halt_w_sb = cpool.tile([128, KT], BF16, tag="haltw")
        nc.sync.dma_start(halt_w_sb[:], hw_e.ap())
        hb_sb = cpool.tile([1, 1], F32, tag="haltb")
        nc.sync.dma_start(hb_sb[:], hb_e.ap())

        bih_sb = cpool.tile([128, GT], F32, tag="bih")
        bhh_sb = cpool.tile([128, GT], F32, tag="bhh")
        wf_sb = cpool.tile([128, GT], F32, tag="wf")
        nc.sync.dma_start(bih_sb[:], bih_e.ap())
        nc.sync.dma_start(bhh_sb[:], bhh_e.ap())
        nc.sync.dma_start(wf_sb[:], wf_e.ap())
        bias_sum = cpool.tile([128, GT], F32, tag="bsum")  # b_ih + b_hh
        nc.vector.tensor_tensor(bias_sum[:], bih_sb[:], bhh_sb[:], ALU.add)
        bias0_sum = cpool.tile([128, GT], F32, tag="b0sum")  # + w_flag (step 0)
        nc.vector.tensor_tensor(bias0_sum[:], bias_sum[:], wf_sb[:], ALU.add)
        bias0_n = cpool.tile([128, GT], F32, tag="b0n")  # b_ih + w_flag (step 0)
        nc.vector.tensor_tensor(bias0_n[:], bih_sb[:], wf_sb[:], ALU.add)

        # ---- persistent per-sample rows ------------------------------
        def row(tag, init=0.0):
            t = cpool.tile([1, B], F32, tag=tag, name=f"row_{tag}")
            nc.gpsimd.memset(t[:], init)
            return t

        still = row("still", 1.0)
        acc_p = row("accp")
        steps_r = row("steps")
        rem_v = row("remv")
        nat_r = row("nat")
        forc_r = row("forc")
        p_sb = row("prow")
        new_acc = row("newacc")
        one_m = row("onem")
        natf = row("natf")
        p_adj = row("padj")
        rem = row("rem")
        sel = row("sel")

        curve_sb = cpool.tile([1, 32], F32, tag="curve")
        nc.gpsimd.memset(curve_sb[:], 0.0)
        count_sb = cpool.tile([1, 1], F32, tag="count")
        nc.gpsimd.memset(count_sb[:], 1.0)

        # ---- DRAM scratch --------------------------------------------
        xw_ts = [dpool.tile([128, B], BF16, tag=f"xw{j}", name=f"xwts{j}")
                 for j in range(GT)]
        acc_ts = [[dpool.tile([128, CH], F32, tag=f"acc{i}_{c}",
                              name=f"accts{i}_{c}")
                   for c in range(NCH)] for i in range(KT)]

        # ---- input projection: xw_ts[jt] = (x @ W_ih[:, :IN].T)^T ----
        with tc.tile_pool(name="xtp", bufs=1) as xtpool:
            xT = xtpool.tile([128, IT * B], BF16, tag="xT")
            nc.sync.dma_start(xT[:], xt_e.ap())
            proj_order = [g * KT + i for i in range(KT) for g in range(3)]
            for jt in proj_order:
                wT = wpool.tile([128, IT * 128], BF16, tag="wblk")
                nc.sync.dma_start(wT[:], wih_e.ap()[jt])
                xst = xwpool.tile([128, B], BF16, tag="xwst")
                for c in range(NCH):
                    px = pspool.tile([128, CH], F32, tag="ps")
                    for it in range(IT):
                        nc.tensor.matmul(
                            px[:],
                            wT[:, it * 128:(it + 1) * 128],
                            xT[:, it * B + c * CH: it * B + c * CH + CH],
                            start=(it == 0), stop=(it == IT - 1),
                        )
                    nc.vector.tensor_copy(xst[:, c * CH:(c + 1) * CH], px[:])
                nc.sync.dma_start(xw_ts[jt][:], xst[:])

        # ---- h ping-pong ---------------------------------------------
        hpool = st.enter_context(tc.tile_pool(name="hbuf", bufs=1))
        h_a = hpool.tile([128, KT * B], BF16, tag="ha")
        h_b = hpool.tile([128, KT * B], BF16, tag="hb")
        hbufs = [h_a, h_b]

        # ---- halting tail (shared by all steps) ----------------------
        def acc_pass(t, h_nxt):
            # acc_state accumulation: acc += p_adj * h_nxt  (HWDGE RMW)
            for c in range(NCH):
                pP = pspool.tile([128, CH], F32, tag="ps")
                nc.tensor.matmul(pP[:], ones_col[:],
                                 p_adj[0:1, c * CH:(c + 1) * CH],
                                 start=True, stop=True)
                for i in range(KT):
                    tmp = gpool.tile([128, CH], F32, tag="acctmp")
                    nc.vector.tensor_tensor(
                        tmp[:], pP[:],
                        h_nxt[:, i * B + c * CH: i * B + c * CH + CH],
                        ALU.mult)
                    if t == 0:
                        nc.sync.dma_start(acc_ts[i][c][:], tmp[:])
                    else:
                        a_in = gpool.tile([128, CH], F32, tag="accin", bufs=4)
                        nc.sync.dma_start(a_in[:], acc_ts[i][c][:])
                        a_new = gpool.tile([128, CH], F32, tag="accnew",
                                           bufs=2)
                        eng = nc.vector if i % 2 == 0 else nc.gpsimd
                        eng.tensor_tensor(a_new[:], a_in[:], tmp[:], ALU.add)
                        nc.sync.dma_start(acc_ts[i][c][:], a_new[:])

        def halt_tail(t, h_nxt, ps_p, defer_acc=False):
            for c in range(NCH):
                nc.scalar.activation(p_sb[0:1, c * CH:(c + 1) * CH],
                                     ps_p[c][:], AF.Sigmoid, bias=hb_sb[:])
            nc.vector.tensor_tensor(new_acc[:], acc_p[:], p_sb[:], ALU.add)
            nc.gpsimd.tensor_scalar(one_m[:], acc_p[:], -1.0, 1.0,
                                    ALU.mult, ALU.add)
            nc.vector.tensor_scalar(natf[:], new_acc[:], float(THRESH),
                                    None, ALU.is_ge)
            nc.vector.tensor_tensor(natf[:], natf[:], still[:], ALU.mult)
            if t == T - 1:
                # forced halt of everyone still running
                nc.vector.tensor_tensor(p_adj[:], still[:], one_m[:], ALU.mult)
                nc.vector.tensor_copy(rem[:], p_adj[:])
                nc.vector.tensor_tensor(sel[:], still[:], natf[:],
                                        ALU.subtract)
                nc.vector.tensor_tensor(forc_r[:], forc_r[:], sel[:], ALU.add)
            else:
                # where(natf, one_m, p) == p + natf * (one_m - p)
                nc.vector.tensor_tensor(sel[:], one_m[:], p_sb[:],
                                        ALU.subtract)
                nc.vector.tensor_tensor(sel[:], natf[:], sel[:], ALU.mult)
                nc.vector.tensor_tensor(sel[:], sel[:], p_sb[:], ALU.add)
                nc.vector.tensor_tensor(p_adj[:], sel[:], still[:], ALU.mult)
                nc.vector.tensor_tensor(rem[:], natf[:], one_m[:], ALU.mult)
            nc.vector.scalar_tensor_tensor(
                acc_p[:], acc_p[:], 0.0, p_adj[:], ALU.add, ALU.add,
                accum_out=curve_sb[0:1, t:t + 1],
            )
            nc.gpsimd.tensor_tensor(steps_r[:], steps_r[:], still[:], ALU.add)
            nc.gpsimd.tensor_tensor(rem_v[:], rem_v[:], rem[:], ALU.add)
            nc.gpsimd.tensor_tensor(nat_r[:], nat_r[:], natf[:], ALU.add)
            if t == T - 1:
                nc.gpsimd.memset(still[:], 0.0)
            else:
                nc.vector.tensor_tensor(still[:], still[:], natf[:],
                                        ALU.subtract)
                nc.vector.tensor_reduce(count_sb[:], still[:], AX.X, ALU.add)
            if not defer_acc:
                acc_pass(t, h_nxt)

        # ---- step 0 (h=0: gates come from xW + biases only) ----------
        h_nxt = hbufs[1]
        ps_p0 = [pppool.tile([1, CH], F32, tag="pp", name=f"psp0_{c}")
                 for c in range(NCH)]
        for i in range(KT):
            xw_r = xwpool.tile([128, B], BF16, tag="xw")
            xw_z = xwpool.tile([128, B], BF16, tag="xw")
            xw_n = xwpool.tile([128, B], BF16, tag="xw")
            nc.sync.dma_start(xw_r[:], xw_ts[i][:])
            nc.sync.dma_start(xw_z[:], xw_ts[KT + i][:])
            nc.sync.dma_start(xw_n[:], xw_ts[2 * KT + i][:])
            hs0 = slice(i * B, (i + 1) * B)
            r0 = gpool.tile([128, B], BF16, tag="r")
            nc.scalar.activation(r0[:], xw_r[:], AF.Sigmoid,
                                 bias=bias0_sum[:, i:i + 1])
            z0 = gpool.tile([128, B], BF16, tag="z")
            nc.scalar.activation(z0[:], xw_z[:], AF.Sigmoid,
                                 bias=bias0_sum[:, KT + i:KT + i + 1])
            u0 = gpool.tile([128, B], BF16, tag="u")
            nc.vector.scalar_tensor_tensor(
                u0[:], r0[:], bhh_sb[:, 2 * KT + i:2 * KT + i + 1],
                xw_n[:], ALU.mult, ALU.add)
            n0 = gpool.tile([128, B], BF16, tag="n")
            nc.scalar.activation(n0[:], u0[:], AF.Tanh,
                                 bias=bias0_n[:, 2 * KT + i:2 * KT + i + 1])
            e0 = gpool.tile([128, B], BF16, tag="e")
            nc.vector.tensor_tensor(e0[:], z0[:], n0[:], ALU.mult)
            nc.vector.tensor_tensor(h_nxt[:, hs0], n0[:], e0[:], ALU.subtract)
            for c in range(NCH):
                nc.tensor.matmul(
                    ps_p0[c][:], halt_w_sb[:, i:i + 1],
                    h_nxt[:, i * B + c * CH: i * B + (c + 1) * CH],
                    start=(i == 0), stop=(i == KT - 1))
        halt_tail(0, h_nxt, ps_p0)

        # ---- steps 1..n_steps-1 with early exit ----------------------
        def step_body(t):
            h_cur = hbufs[t % 2]
            h_nx = hbufs[(t + 1) % 2]
            ps_p_t = [pppool.tile([1, CH], F32, tag="pp", name=f"pspt{t}_{c}")
                      for c in range(NCH)]
            for i in range(KT):
                jr, jz, jn = i, KT + i, 2 * KT + i
                w_r = wpool.tile([128, KT * 128], BF16, tag="wblk")
                w_z = wpool.tile([128, KT * 128], BF16, tag="wblk")
                w_n = wpool.tile([128, KT * 128], BF16, tag="wblk")
                nc.sync.dma_start(w_r[:], whh_e.ap()[jr])
                nc.sync.dma_start(w_z[:], whh_e.ap()[jz])
                nc.sync.dma_start(w_n[:], whh_e.ap()[jn])
                xw_r = xwpool.tile([128, B], BF16, tag="xw")
                xw_z = xwpool.tile([128, B], BF16, tag="xw")
                xw_n = xwpool.tile([128, B], BF16, tag="xw")
                nc.sync.dma_start(xw_r[:], xw_ts[jr][:])
                nc.sync.dma_start(xw_z[:], xw_ts[jz][:])
                nc.sync.dma_start(xw_n[:], xw_ts[jn][:])
                for c in range(NCH):
                    sl = slice(c * CH, (c + 1) * CH)
                    hsl = slice(i * B + c * CH, i * B + (c + 1) * CH)
                    ps_r = pspool.tile([128, CH], F32, tag="ps")
                    ps_z = pspool.tile([128, CH], F32, tag="ps")
                    ps_n = pspool.tile([128, CH], F32, tag="ps")
                    for kt in range(KT):
                        rh = h_cur[:, kt * B + c * CH: kt * B + (c + 1) * CH]
                        nc.tensor.matmul(
                            ps_r[:], w_r[:, kt * 128:(kt + 1) * 128], rh,
                            start=(kt == 0), stop=False)
                        nc.tensor.matmul(
                            ps_z[:], w_z[:, kt * 128:(kt + 1) * 128], rh,
                            start=(kt == 0), stop=False)
                        nc.tensor.matmul(
                            ps_n[:], w_n[:, kt * 128:(kt + 1) * 128], rh,
                            start=(kt == 0), stop=(kt == KT - 1))
                    nc.tensor.matmul(ps_r[:], ident_b[:], xw_r[:, sl],
                                     start=False, stop=True)
                    nc.tensor.matmul(ps_z[:], ident_b[:], xw_z[:, sl],
                                     start=False, stop=True)
                    r_t = gpool.tile([128, CH], BF16, tag="r")
                    nc.scalar.activation(r_t[:], ps_r[:], AF.Sigmoid,
                                         bias=bias_sum[:, jr:jr + 1])
                    z_t = gpool.tile([128, CH], BF16, tag="z")
                    nc.scalar.activation(z_t[:], ps_z[:], AF.Sigmoid,
                                         bias=bias_sum[:, jz:jz + 1])
                    u_t = gpool.tile([128, CH], BF16, tag="u")
                    nc.vector.scalar_tensor_tensor(
                        u_t[:], ps_n[:], bhh_sb[:, jn:jn + 1], r_t[:],
                        ALU.add, ALU.mult)
                    v_t = gpool.tile([128, CH], BF16, tag="v")
                    nc.vector.tensor_tensor(v_t[:], u_t[:], xw_n[:, sl],
                                            ALU.add)
                    n_t = gpool.tile([128, CH], BF16, tag="n")
                    nc.scalar.activation(n_t[:], v_t[:], AF.Tanh,
                                         bias=bih_sb[:, jn:jn + 1])
                    d_t = gpool.tile([128, CH], BF16, tag="d")
                    nc.vector.tensor_tensor(d_t[:], h_cur[:, hsl], n_t[:],
                                            ALU.subtract)
                    e_t = gpool.tile([128, CH], BF16, tag="e")
                    nc.vector.tensor_tensor(e_t[:], z_t[:], d_t[:], ALU.mult)
                    nc.vector.tensor_tensor(h_nx[:, hsl], n_t[:], e_t[:],
                                            ALU.add)
                    nc.tensor.matmul(ps_p_t[c][:], halt_w_sb[:, i:i + 1],
                                     h_nx[:, hsl],
                                     start=(i == 0), stop=(i == KT - 1))
            halt_tail(t, h_nx, ps_p_t)

        def load_count():
            return nc.values_load(
                count_sb[:].bitcast(I32), min_val=0, max_val=2 ** 30,
                skip_runtime_bounds_check=True)

        if n_steps > 1:
            c1 = load_count()
            with tc.If(c1 > 0):
                step_body(1)
        if n_steps > 2:
            c2 = load_count()
            with tc.If(c2 > 0):
                for t in range(2, n_steps):
                    ct = load_count()
                    with tc.If(ct > 0):
                        step_body(t)

        # ---- final outputs -------------------------------------------
        for i in range(KT):
            for c in range(NCH):
                nc.sync.dma_start(
                    acc_e.ap()[i * 128:(i + 1) * 128, c * CH:(c + 1) * CH],
                    acc_ts[i][c][:])
        nc.sync.dma_start(stats_e.ap()[0:1, :], steps_r[:])
        nc.sync.dma_start(stats_e.ap()[1:2, :], rem_v[:])
        nc.sync.dma_start(stats_e.ap()[2:3, :], nat_r[:])
        nc.sync.dma_start(stats_e.ap()[3:4, :], forc_r[:])
        nc.sync.dma_start(curve_e.ap()[:, :], curve_sb[:])


# --------------------------------------------------------------- runner ----
_CACHE = {}


def _get_nc(n_steps=T):
    if n_steps not in _CACHE:
        _CACHE[n_steps] = _build(n_steps)[0]
    return _CACHE[n_steps]


def _marshal(inputs):
    """Host-side input marshaling: shard x, pre-transpose / tile-block /
    bf16-cast the replicated weights into the layouts the device consumes."""
    bf = ml_dtypes.bfloat16
    x = np.asarray(inputs["input_tensor"], dtype=np.float32)
    w_ih = np.asarray(inputs["weight_ih"], dtype=np.float32)
    w_hh = np.asarray(inputs["weight_hh"], dtype=np.float32)
    b_ih = np.asarray(inputs["bias_ih"], dtype=np.float32)
    b_hh = np.asarray(inputs["bias_hh"], dtype=np.float32)
    halt_w = np.asarray(inputs["halt_w"], dtype=np.float32)
    halt_b = np.asarray(inputs["halt_b"], dtype=np.float32)

    # w_ih_t[jt, p, it*128+j'] = w_ih[jt*128+j', it*128+p]
    wih_blk = w_ih[:, :IN].reshape(GT, 128, IT, 128)  # [jt, j', it, p]
    wih_blk = np.ascontiguousarray(
        wih_blk.transpose(0, 3, 2, 1).reshape(GT, 128, IT * 128)
    ).astype(bf)

    # w_hh_t[jt, p, kt*128+j'] = w_hh[jt*128+j', kt*128+p]
    whh_blk = w_hh.reshape(GT, 128, KT, 128)  # [jt, j', kt, p]
    whh_blk = np.ascontiguousarray(
        whh_blk.transpose(0, 3, 2, 1).reshape(GT, 128, KT * 128)
    ).astype(bf)

    bih_p = np.ascontiguousarray(b_ih.reshape(GT, 128).T)
    bhh_p = np.ascontiguousarray(b_hh.reshape(GT, 128).T)
    wf_p = np.ascontiguousarray(w_ih[:, IN].reshape(GT, 128).T)
    hw_p = np.ascontiguousarray(halt_w.reshape(KT, 128).T.astype(bf))
    hb_p = np.ascontiguousarray(halt_b.reshape(1, 1))

    base = {
        "w_ih_t": wih_blk, "w_hh_t": whh_blk,
        "bih_p": bih_p, "bhh_p": bhh_p, "wf_p": wf_p,
        "hw_p": hw_p, "hb_p": hb_p,
    }
    in_maps = []
    for ci in range(N_CORES):
        xs = x[ci * B:(ci + 1) * B]  # [B, IN]
        # x_t[p, it*B + b] = x[b, it*128+p]
        xt = np.ascontiguousarray(
            xs.reshape(B, IT, 128).transpose(2, 1, 0).reshape(128, IT * B)
        ).astype(bf)
        in_maps.append(dict(base, x_t=xt))
    return in_maps


def run_device(inputs, n_steps=T, trace=False):
    from concourse.bass_utils import run_bass_kernel_spmd

    nc = _get_nc(n_steps)
    in_maps = _marshal(inputs)
    return run_bass_kernel_spmd(nc, in_maps, core_ids=list(range(N_CORES)),
                                trace=trace)


def combine(results):
    """Host-side unshard + tiny final statistics."""
    accs, steps_l, rem_l, curves, tmaxes = [], [], [], [], []
    nat_s = forc_s = 0.0
    for ci in range(N_CORES):
        r = results[ci]
        accs.append(np.ascontiguousarray(r["acc_t"].T))
        stats = r["stats"]
        steps_l.append(stats[0])
        rem_l.append(stats[1])
        nat_s += float(stats[2].sum(dtype=np.float64))
        forc_s += float(stats[3].sum(dtype=np.float64))
        curves.append(r["curve"][0, :T].copy())
        tmaxes.append(int(stats[0].max()))

    acc_state = np.concatenate(accs, axis=0)
    steps = np.concatenate(steps_l)
    rem_v = np.concatenate(rem_l)

    # pad each core's curve past its last executed step with its final value
    curve = np.zeros(T, dtype=np.float64)
    for ci in range(N_CORES):
        c = curves[ci].astype(np.float64)
        tm = max(tmaxes[ci], 1)
        c[tm:] = c[tm - 1]
        curve += c
    curve = (curve / B_FULL).astype(np.float32)

    ponder = (steps + rem_v) * TIME_PENALTY
    final_ponder = np.float32(ponder.mean(dtype=np.float64))
    remainder_mean = np.float32(rem_v.mean(dtype=np.float64))
    remainder_std = np.float32(rem_v.std(dtype=np.float64))
    natural_ratio = np.float32(nat_s / B_FULL)
    forced_ratio = np.float32(forc_s / B_FULL)
    p50 = np.float32(np.quantile(steps.astype(np.float64), 0.5))
    p90 = np.float32(np.quantile(steps.astype(np.float64), 0.9))
    return (acc_state, final_ponder, steps, remainder_mean, remainder_std,
            natural_ratio, forced_ratio, p50, p90, curve)


def kernel(**inputs):
    res = run_device(inputs, n_steps=T, trace=False)
    return combine(res.results)


if __name__ == "__main__":
    import os
    import time

    t0 = time.time()
    nc, nfix = _build(int(os.environ.get("NSTEPS", T)))
    print(f"built ok in {time.time() - t0:.1f}s, waitsplit fixes: {nfix}")


# revision 20
# speedup vs baseline: 5.7527x; 1.0382x over previous
"""AdaptiveRNNCell (ACT-halting GRU) Trainium2 kernel, 8-core data-parallel.

B=8192 batch sharded 1024/core; GRU weights replicated. All per-step state is
kept transposed [H-on-partitions, batch-on-free] so the recurrent matmul
h @ W_hh^T needs no per-step transposes. Weights are shipped pre-transposed /
tile-blocked in bf16 (host-side input marshaling), so the device does only
the input projection and the recurrent steps. Halting is per-sample; steps
1..19 are wrapped in runtime If(any_still_running) so the kernel stops
computing once every sample has halted (the torch module breaks early; with
halt bias 1.0 nearly everything halts after ~2 steps). Final scalar
statistics (means / quantiles / curve padding) are reduced on host from tiny
per-core vectors.
"""

import sys

for _p in ("/root/.axon_site/_ro/trn_rl_repo", "/opt/trn_rl_repo"):
    if _p not in sys.path:
        sys.path.append(_p)

import ml_dtypes
import numpy as np

import concourse.bass as bass
import concourse.mybir as mybir
import concourse.tile as tile
from concourse.masks import make_identity

N_CORES = 8
B_FULL, IN, H = 8192, 1024, 2048
B = B_FULL // N_CORES  # 1024 per core
G3 = 3 * H  # 6144
KT = H // 128  # 16 h tiles
GT = G3 // 128  # 48 gate tiles
IT = IN // 128  # 8 input tiles
CH = 512  # matmul moving chunk
NCH = B // CH  # 2
T = 20
THRESH = np.float32(1.0 - 0.01)
TIME_PENALTY = np.float32(0.001)

F32 = mybir.dt.float32
BF16 = mybir.dt.bfloat16
I32 = mybir.dt.int32
AF = mybir.ActivationFunctionType
ALU = mybir.AluOpType
AX = mybir.AxisListType


# ---------------------------------------------------------------- shims ----
def _patch_tile_drain():
    """walrus here rejects >1 sem wait on CTRL instructions: split the tile
    kernel-tail drain's waits across single-wait NOPs."""
    if getattr(tile.TileContext, "_drain_patched", False):
        return
    from concourse.vector_clock import ScopedClock

    def _patched(self, tick_clock, wait_clock):
        nc = self.nc
        drain_inst = nc.sync.drain()
        wait_clock.add_sem_waits(
            drain_inst.ins, ScopedClock({None: tick_clock.global_clock})
        )
        waits = list(drain_inst.ins.sync_info.on_wait)
        if len(waits) > 1:
            drain_inst.ins.sync_info = mybir.SyncInfo(
                on_wait=waits[:1], on_update=[]
            )
            for w in waits[1:]:
                nop = nc.sync.nop(nofuse=True)
                nop.ins.sync_info = mybir.SyncInfo(on_wait=[w], on_update=[])
        nc.all_engine_barrier()
        assert self.sems is not None
        popped = nc._tile_sem_poison_stack.pop()
        assert popped is self._sem_poison
        nc.clear_and_free_semaphores(list(self.sems.allocated().values()))
        nc.all_engine_barrier()

    tile.TileContext._drain_and_barrier = _patched
    tile.TileContext._drain_patched = True


def _split_excess_waits(nc, limit=1, max_upd=63):
    """walrus here caps sem waits per instruction and only supports small
    sem increments on compute instructions. Hoist excess waits onto
    same-engine NOPs and oversized sem-add updates onto EventSemaphore
    instructions emitted right after the owner."""
    n_fixed = 0
    for fn in nc.m.functions:
        for blk in fn.blocks:
            changed = False
            new_list = []
            for inst in blk.instructions:
                si = inst.sync_info
                waits = list(si.on_wait) if si is not None else []
                upds = list(si.on_update) if si is not None else []
                big_upds = [
                    u for u in upds
                    if getattr(u, "update_mode", "") == "sem-add-imm"
                    and getattr(u, "update_value", 0) > 1
                    and inst.opcode not in ("EventSemaphore", "ISA",
                                            "DMACopy", "Drain", "NoOp")
                ]
                if len(waits) > limit or big_upds:
                    hoist, keep = waits[:-limit], waits[-limit:]
                    for w in hoist:
                        n_fixed += 1
                        nop = mybir.InstNoOp(
                            name=f"waitsplit-{n_fixed}-{inst.name}", ins=[], outs=[]
                        )
                        nop.engine = inst.engine
                        nop.sync_info = mybir.SyncInfo(on_wait=[w], on_update=[])
                        new_list.append(nop)
                    keep_upds = [u for u in upds if u not in big_upds]
                    tail = []
                    for u in big_upds:
                        left = u.update_value - 1
                        ku = mybir.SyncUpdate(
                            ant_name=u.ant_name, id=u.id,
                            sync_type=u.sync_type,
                            update_mode="sem-inc", update_value=1)
                        keep_upds.append(ku)
                        while left > 0:
                            n_fixed += 1
                            cu = mybir.SyncUpdate(
                                ant_name=u.ant_name, id=u.id,
                                sync_type=u.sync_type,
                                update_mode="sem-add-imm",
                                update_value=min(left, max_upd))
                            ev = mybir.InstEventSemaphore(
                                name=f"updsplit-{n_fixed}-{inst.name}",
                                ins=[], outs=[])
                            ev.engine = inst.engine
                            ev.sync_info = mybir.SyncInfo(on_wait=[],
                                                          on_update=[cu])
                            tail.append(ev)
                            left -= max_upd
                    inst.sync_info = mybir.SyncInfo(
                        on_wait=keep, on_update=keep_upds
                    )
                    new_list.append(inst)
                    new_list.extend(tail)
                    changed = True
                else:
                    new_list.append(inst)
            if changed:
                blk.instructions = new_list
    return n_fixed


# ------------------------------------------------------------- builder ----
def _build(n_steps=T):
    _patch_tile_drain()
    nc = bass.Bass("TRN2", target_bir_lowering=False, debug=False,
                   num_devices=N_CORES)

    # host-marshaled inputs (pre-transposed / tile-blocked / pre-cast)
    xt_e = nc.dram_tensor("x_t", [128, IT * B], BF16, kind="ExternalInput")
    wih_e = nc.dram_tensor("w_ih_t", [GT, 128, IT * 128], BF16,
                           kind="ExternalInput")
    whh_e = nc.dram_tensor("w_hh_t", [GT, 128, KT * 128], BF16,
                           kind="ExternalInput")
    bih_e = nc.dram_tensor("bih_p", [128, GT], F32, kind="ExternalInput")
    bhh_e = nc.dram_tensor("bhh_p", [128, GT], F32, kind="ExternalInput")
    wf_e = nc.dram_tensor("wf_p", [128, GT], F32, kind="ExternalInput")
    hw_e = nc.dram_tensor("hw_p", [128, KT], BF16, kind="ExternalInput")
    hb_e = nc.dram_tensor("hb_p", [1, 1], F32, kind="ExternalInput")

    acc_e = nc.dram_tensor("acc_t", [H, B], F32, kind="ExternalOutput")
    stats_e = nc.dram_tensor("stats", [4, B], F32, kind="ExternalOutput")
    curve_e = nc.dram_tensor("curve", [1, 32], F32, kind="ExternalOutput")

    with tile.TileContext(nc) as tc:
        _body(nc, tc, n_steps, xt_e, wih_e, whh_e, bih_e, bhh_e, wf_e,
              hw_e, hb_e, acc_e, stats_e, curve_e)

    nfix = _split_excess_waits(nc, limit=1)
    return nc, nfix


def _body(nc, tc, n_steps, xt_e, wih_e, whh_e, bih_e, bhh_e, wf_e,
          hw_e, hb_e, acc_e, stats_e, curve_e):
    from contextlib import ExitStack

    with ExitStack() as st:
        cpool = st.enter_context(tc.tile_pool(name="const", bufs=1))
        wpool = st.enter_context(tc.tile_pool(name="wstream", bufs=6))
        xwpool = st.enter_context(tc.tile_pool(name="xwstream", bufs=4))
        gpool = st.enter_context(tc.tile_pool(name="gates", bufs=2))
        pspool = st.enter_context(tc.tile_pool(name="ps", bufs=6, space="PSUM"))
        pppool = st.enter_context(tc.tile_pool(name="pp", bufs=2, space="PSUM"))
        dpool = st.enter_context(tc.tile_pool(name="dram", bufs=1, space="DRAM"))

        # ---- constants -----------------------------------------------
        ident_b = cpool.tile([128, 128], BF16, tag="identb")
        make_identity(nc, ident_b[:])
        ones_col = cpool.tile([1, 128], F32, tag="ones")
        nc.gpsimd.memset(ones_col[:], 1.0)

        halt_w_sb = cpool.tile([128, KT], BF16, tag="haltw")
        nc.sync.dma_start(halt_w_sb[:], hw_e.ap())
        hb_sb = cpool.tile([1, 1], F32, tag="haltb")
        nc.sync.dma_start(hb_sb[:], hb_e.ap())

        bih_sb = cpool.tile([128, GT], F32, tag="bih")
        bhh_sb = cpool.tile([128, GT], F32, tag="bhh")
        wf_sb = cpool.tile([128, GT], F32, tag="wf")
        nc.sync.dma_start(bih_sb[:], bih_e.ap())
        nc.sync.dma_start(bhh_sb[:], bhh_e.ap())
        nc.sync.dma_start(wf_sb[:], wf_e.ap())
        bias_sum = cpool.tile([128, GT], F32, tag="bsum")  # b_ih + b_hh
        nc.vector.tensor_tensor(bias_sum[:], bih_sb[:], bhh_sb[:], ALU.add)
        bias0_sum = cpool.tile([128, GT], F32, tag="b0sum")  # + w_flag (step 0)
        nc.vector.tensor_tensor(bias0_sum[:], bias_sum[:], wf_sb[:], ALU.add)
        bias0_n = cpool.tile([128, GT], F32, tag="b0n")  # b_ih + w_flag (step 0)
        nc.vector.tensor_tensor(bias0_n[:], bih_sb[:], wf_sb[:], ALU.add)

        # ---- persistent per-sample rows ------------------------------
        def row(tag, init=0.0):
            t = cpool.tile([1, B], F32, tag=tag, name=f"row_{tag}")
            nc.gpsimd.memset(t[:], init)
            return t

        still = row("still", 1.0)
        acc_p = row("accp")
        steps_r = row("steps")
        rem_v = row("remv")
        nat_r = row("nat")
        forc_r = row("forc")
        p_sb = row("prow")
        new_acc = row("newacc")
        one_m = row("onem")
        natf = row("natf")
        p_adj = row("padj")
        rem = row("rem")
        sel = row("sel")

        curve_sb = cpool.tile([1, 32], F32, tag="curve")
        nc.gpsimd.memset(curve_sb[:], 0.0)
        count_sb = cpool.tile([1, 1], F32, tag="count")
        nc.gpsimd.memset(count_sb[:], 1.0)

        # ---- DRAM scratch --------------------------------------------
        xw_ts = [dpool.tile([128, B], BF16, tag=f"xw{j}", name=f"xwts{j}")
                 for j in range(GT)]
        acc_ts = [[dpool.tile([128, CH], F32, tag=f"acc{i}_{c}",
                              name=f"accts{i}_{c}")
                   for c in range(NCH)] for i in range(KT)]

        # ---- input projection: xw_ts[jt] = (x @ W_ih[:, :IN].T)^T ----
        with tc.tile_pool(name="xtp", bufs=1) as xtpool:
            xT = xtpool.tile([128, IT * B], BF16, tag="xT")
            nc.sync.dma_start(xT[:], xt_e.ap())
            proj_order = [g * KT + i for i in range(KT) for g in range(3)]
            for jt in proj_order:
                wT = wpool.tile([128, IT * 128], BF16, tag="wblk")
                nc.sync.dma_start(wT[:], wih_e.ap()[jt])
                xst = xwpool.tile([128, B], BF16, tag="xwst")
                for c in range(NCH):
                    px = pspool.tile([128, CH], F32, tag="ps")
                    for it in range(IT):
                        nc.tensor.matmul(
                            px[:],
                            wT[:, it * 128:(it + 1) * 128],
                            xT[:, it * B + c * CH: it * B + c * CH + CH],
                            start=(it == 0), stop=(it == IT - 1),
                        )
                    nc.vector.tensor_copy(xst[:, c * CH:(c + 1) * CH], px[:])
                nc.sync.dma_start(xw_ts[jt][:], xst[:])

        # ---- h ping-pong ---------------------------------------------
        hpool = st.enter_context(tc.tile_pool(name="hbuf", bufs=1))
        h_a = hpool.tile([128, KT * B], BF16, tag="ha")
        h_b = hpool.tile([128, KT * B], BF16, tag="hb")
        hbufs = [h_a, h_b]

        # ---- halting tail (shared by all steps) ----------------------
        def acc_pass(t, h_nxt):
            # acc_state accumulation: acc += p_adj * h_nxt  (HWDGE RMW)
            for c in range(NCH):
                pP = pspool.tile([128, CH], F32, tag="ps")
                nc.tensor.matmul(pP[:], ones_col[:],
                                 p_adj[0:1, c * CH:(c + 1) * CH],
                                 start=True, stop=True)
                for i in range(KT):
                    tmp = gpool.tile([128, CH], F32, tag="acctmp")
                    nc.vector.tensor_tensor(
                        tmp[:], pP[:],
                        h_nxt[:, i * B + c * CH: i * B + c * CH + CH],
                        ALU.mult)
                    if t == 0:
                        nc.sync.dma_start(acc_ts[i][c][:], tmp[:])
                    else:
                        a_in = gpool.tile([128, CH], F32, tag="accin", bufs=4)
                        nc.sync.dma_start(a_in[:], acc_ts[i][c][:])
                        a_new = gpool.tile([128, CH], F32, tag="accnew",
                                           bufs=2)
                        eng = nc.vector if i % 2 == 0 else nc.gpsimd
                        eng.tensor_tensor(a_new[:], a_in[:], tmp[:], ALU.add)
                        nc.sync.dma_start(acc_ts[i][c][:], a_new[:])

        def halt_tail(t, h_nxt, ps_p, defer_acc=False):
            for c in range(NCH):
                nc.scalar.activation(p_sb[0:1, c * CH:(c + 1) * CH],
                                     ps_p[c][:], AF.Sigmoid, bias=hb_sb[:])
            nc.vector.tensor_tensor(new_acc[:], acc_p[:], p_sb[:], ALU.add)
            nc.gpsimd.tensor_scalar(one_m[:], acc_p[:], -1.0, 1.0,
                                    ALU.mult, ALU.add)
            nc.vector.tensor_scalar(natf[:], new_acc[:], float(THRESH),
                                    None, ALU.is_ge)
            nc.vector.tensor_tensor(natf[:], natf[:], still[:], ALU.mult)
            if t == T - 1:
                # forced halt of everyone still running
                nc.vector.tensor_tensor(p_adj[:], still[:], one_m[:], ALU.mult)
                nc.vector.tensor_copy(rem[:], p_adj[:])
                nc.vector.tensor_tensor(sel[:], still[:], natf[:],
                                        ALU.subtract)
                nc.vector.tensor_tensor(forc_r[:], forc_r[:], sel[:], ALU.add)
            else:
                # where(natf, one_m, p) == p + natf * (one_m - p)
                nc.vector.tensor_tensor(sel[:], one_m[:], p_sb[:],
                                        ALU.subtract)
                nc.vector.tensor_tensor(sel[:], natf[:], sel[:], ALU.mult)
                nc.vector.tensor_tensor(sel[:], sel[:], p_sb[:], ALU.add)
                nc.vector.tensor_tensor(p_adj[:], sel[:], still[:], ALU.mult)
                nc.vector.tensor_tensor(rem[:], natf[:], one_m[:], ALU.mult)
            nc.vector.scalar_tensor_tensor(
                acc_p[:], acc_p[:], 0.0, p_adj[:], ALU.add, ALU.add,
                accum_out=curve_sb[0:1, t:t + 1],
            )
            nc.gpsimd.tensor_tensor(steps_r[:], steps_r[:], still[:], ALU.add)
            nc.gpsimd.tensor_tensor(rem_v[:], rem_v[:], rem[:], ALU.add)
            nc.gpsimd.tensor_tensor(nat_r[:], nat_r[:], natf[:], ALU.add)
            if t == T - 1:
                nc.gpsimd.memset(still[:], 0.0)
            else:
                nc.vector.tensor_tensor(still[:], still[:], natf[:],
                                        ALU.subtract)
                nc.vector.tensor_reduce(count_sb[:], still[:], AX.X, ALU.add)
            if not defer_acc:
                acc_pass(t, h_nxt)

        # ---- step 0 (h=0: gates come from xW + biases only) ----------
        h_nxt = hbufs[1]
        ps_p0 = [pppool.tile([1, CH], F32, tag="pp", name=f"psp0_{c}")
                 for c in range(NCH)]
        for i in range(KT):
            xw_r = xwpool.tile([128, B], BF16, tag="xw")
            xw_z = xwpool.tile([128, B], BF16, tag="xw")
            xw_n = xwpool.tile([128, B], BF16, tag="xw")
            nc.sync.dma_start(xw_r[:], xw_ts[i][:])
            nc.sync.dma_start(xw_z[:], xw_ts[KT + i][:])
            nc.sync.dma_start(xw_n[:], xw_ts[2 * KT + i][:])
            hs0 = slice(i * B, (i + 1) * B)
            r0 = gpool.tile([128, B], BF16, tag="r")
            nc.scalar.activation(r0[:], xw_r[:], AF.Sigmoid,
                                 bias=bias0_sum[:, i:i + 1])
            z0 = gpool.tile([128, B], BF16, tag="z")
            nc.scalar.activation(z0[:], xw_z[:], AF.Sigmoid,
                                 bias=bias0_sum[:, KT + i:KT + i + 1])
            u0 = gpool.tile([128, B], BF16, tag="u")
            nc.vector.scalar_tensor_tensor(
                u0[:], r0[:], bhh_sb[:, 2 * KT + i:2 * KT + i + 1],
                xw_n[:], ALU.mult, ALU.add)
            n0 = gpool.tile([128, B], BF16, tag="n")
            nc.scalar.activation(n0[:], u0[:], AF.Tanh,
                                 bias=bias0_n[:, 2 * KT + i:2 * KT + i + 1])
            e0 = gpool.tile([128, B], BF16, tag="e")
            nc.vector.tensor_tensor(e0[:], z0[:], n0[:], ALU.mult)
            nc.vector.tensor_tensor(h_nxt[:, hs0], n0[:], e0[:], ALU.subtract)
            for c in range(NCH):
                nc.tensor.matmul(
                    ps_p0[c][:], halt_w_sb[:, i:i + 1],
                    h_nxt[:, i * B + c * CH: i * B + (c + 1) * CH],
                    start=(i == 0), stop=(i == KT - 1))
        halt_tail(0, h_nxt, ps_p0, defer_acc=True)

        # ---- steps 1..n_steps-1 with early exit ----------------------
        def step_body(t):
            h_cur = hbufs[t % 2]
            h_nx = hbufs[(t + 1) % 2]
            ps_p_t = [pppool.tile([1, CH], F32, tag="pp", name=f"pspt{t}_{c}")
                      for c in range(NCH)]
            for i in range(KT):
                jr, jz, jn = i, KT + i, 2 * KT + i
                w_r = wpool.tile([128, KT * 128], BF16, tag="wblk")
                w_z = wpool.tile([128, KT * 128], BF16, tag="wblk")
                w_n = wpool.tile([128, KT * 128], BF16, tag="wblk")
                nc.sync.dma_start(w_r[:], whh_e.ap()[jr])
                nc.sync.dma_start(w_z[:], whh_e.ap()[jz])
                nc.sync.dma_start(w_n[:], whh_e.ap()[jn])
                xw_r = xwpool.tile([128, B], BF16, tag="xw")
                xw_z = xwpool.tile([128, B], BF16, tag="xw")
                xw_n = xwpool.tile([128, B], BF16, tag="xw")
                nc.sync.dma_start(xw_r[:], xw_ts[jr][:])
                nc.sync.dma_start(xw_z[:], xw_ts[jz][:])
                nc.sync.dma_start(xw_n[:], xw_ts[jn][:])
                for c in range(NCH):
                    sl = slice(c * CH, (c + 1) * CH)
                    hsl = slice(i * B + c * CH, i * B + (c + 1) * CH)
                    ps_r = pspool.tile([128, CH], F32, tag="ps")
                    ps_z = pspool.tile([128, CH], F32, tag="ps")
                    ps_n = pspool.tile([128, CH], F32, tag="ps")
                    for kt in range(KT):
                        rh = h_cur[:, kt * B + c * CH: kt * B + (c + 1) * CH]
                        nc.tensor.matmul(
                            ps_r[:], w_r[:, kt * 128:(kt + 1) * 128], rh,
                            start=(kt == 0), stop=False)
                        nc.tensor.matmul(
                            ps_z[:], w_z[:, kt * 128:(kt + 1) * 128], rh,
                            start=(kt == 0), stop=False)
                        nc.tensor.matmul(
                            ps_n[:], w_n[:, kt * 128:(kt + 1) * 128], rh,
                            start=(kt == 0), stop=(kt == KT - 1))
                    nc.tensor.matmul(ps_r[:], ident_b[:], xw_r[:, sl],
                                     start=False, stop=True)
                    nc.tensor.matmul(ps_z[:], ident_b[:], xw_z[:, sl],
                                     start=False, stop=True)
                    r_t = gpool.tile([128, CH], BF16, tag="r")
                    nc.scalar.activation(r_t[:], ps_r[:], AF.Sigmoid,
                                         bias=bias_sum[:, jr:jr + 1])
                    z_t = gpool.tile([128, CH], BF16, tag="z")
                    nc.scalar.activation(z_t[:], ps_z[:], AF.Sigmoid,
                                         bias=bias_sum[:, jz:jz + 1])
                    u_t = gpool.tile([128, CH], BF16, tag="u")
                    nc.vector.scalar_tensor_tensor(
                        u_t[:], ps_n[:], bhh_sb[:, jn:jn + 1], r_t[:],
                        ALU.add, ALU.mult)
                    v_t = gpool.tile([128, CH], BF16, tag="v")
                    nc.vector.tensor_tensor(v_t[:], u_t[:], xw_n[:, sl],
                                            ALU.add)
                    n_t = gpool.tile([128, CH], BF16, tag="n")
                    nc.scalar.activation(n_t[:], v_t[:], AF.Tanh,
                                         bias=bih_sb[:, jn:jn + 1])
                    d_t = gpool.tile([128, CH], BF16, tag="d")
                    nc.vector.tensor_tensor(d_t[:], h_cur[:, hsl], n_t[:],
                                            ALU.subtract)
                    e_t = gpool.tile([128, CH], BF16, tag="e")
                    nc.vector.tensor_tensor(e_t[:], z_t[:], d_t[:], ALU.mult)
                    nc.vector.tensor_tensor(h_nx[:, hsl], n_t[:], e_t[:],
                                            ALU.add)
                    nc.tensor.matmul(ps_p_t[c][:], halt_w_sb[:, i:i + 1],
                                     h_nx[:, hsl],
                                     start=(i == 0), stop=(i == KT - 1))
            halt_tail(t, h_nx, ps_p_t)

        def load_count():
            return nc.values_load(
                count_sb[:].bitcast(I32), min_val=0, max_val=2 ** 30,
                skip_runtime_bounds_check=True)

        if n_steps > 1:
            c1 = load_count()
            with tc.If(c1 > 0) as cmp1:
                acc_pass(0, hbufs[1])
                step_body(1)
            with cmp1.Else():
                acc_pass(0, hbufs[1])
        else:
            acc_pass(0, hbufs[1])
        if n_steps > 2:
            c2 = load_count()
            with tc.If(c2 > 0):
                for t in range(2, n_steps):
                    ct = load_count()
                    with tc.If(ct > 0):
                        step_body(t)

        # ---- final outputs -------------------------------------------
        for i in range(KT):
            for c in range(NCH):
                nc.sync.dma_start(
                    acc_e.ap()[i * 128:(i + 1) * 128, c * CH:(c + 1) * CH],
                    acc_ts[i][c][:])
        nc.sync.dma_start(stats_e.ap()[0:1, :], steps_r[:])
        nc.sync.dma_start(stats_e.ap()[1:2, :], rem_v[:])
        nc.sync.dma_start(stats_e.ap()[2:3, :], nat_r[:])
        nc.sync.dma_start(stats_e.ap()[3:4, :], forc_r[:])
        nc.sync.dma_start(curve_e.ap()[:, :], curve_sb[:])


# --------------------------------------------------------------- runner ----
_CACHE = {}


def _get_nc(n_steps=T):
    if n_steps not in _CACHE:
        _CACHE[n_steps] = _build(n_steps)[0]
    return _CACHE[n_steps]


def _marshal(inputs):
    """Host-side input marshaling: shard x, pre-transpose / tile-block /
    bf16-cast the replicated weights into the layouts the device consumes."""
    bf = ml_dtypes.bfloat16
    x = np.asarray(inputs["input_tensor"], dtype=np.float32)
    w_ih = np.asarray(inputs["weight_ih"], dtype=np.float32)
    w_hh = np.asarray(inputs["weight_hh"], dtype=np.float32)
    b_ih = np.asarray(inputs["bias_ih"], dtype=np.float32)
    b_hh = np.asarray(inputs["bias_hh"], dtype=np.float32)
    halt_w = np.asarray(inputs["halt_w"], dtype=np.float32)
    halt_b = np.asarray(inputs["halt_b"], dtype=np.float32)

    # w_ih_t[jt, p, it*128+j'] = w_ih[jt*128+j', it*128+p]
    wih_blk = w_ih[:, :IN].reshape(GT, 128, IT, 128)  # [jt, j', it, p]
    wih_blk = np.ascontiguousarray(
        wih_blk.transpose(0, 3, 2, 1).reshape(GT, 128, IT * 128)
    ).astype(bf)

    # w_hh_t[jt, p, kt*128+j'] = w_hh[jt*128+j', kt*128+p]
    whh_blk = w_hh.reshape(GT, 128, KT, 128)  # [jt, j', kt, p]
    whh_blk = np.ascontiguousarray(
        whh_blk.transpose(0, 3, 2, 1).reshape(GT, 128, KT * 128)
    ).astype(bf)

    bih_p = np.ascontiguousarray(b_ih.reshape(GT, 128).T)
    bhh_p = np.ascontiguousarray(b_hh.reshape(GT, 128).T)
    wf_p = np.ascontiguousarray(w_ih[:, IN].reshape(GT, 128).T)
    hw_p = np.ascontiguousarray(halt_w.reshape(KT, 128).T.astype(bf))
    hb_p = np.ascontiguousarray(halt_b.reshape(1, 1))

    base = {
        "w_ih_t": wih_blk, "w_hh_t": whh_blk,
        "bih_p": bih_p, "bhh_p": bhh_p, "wf_p": wf_p,
        "hw_p": hw_p, "hb_p": hb_p,
    }
    in_maps = []
    for ci in range(N_CORES):
        xs = x[ci * B:(ci + 1) * B]  # [B, IN]
        # x_t[p, it*B + b] = x[b, it*128+p]
        xt = np.ascontiguousarray(
            xs.reshape(B, IT, 128).transpose(2, 1, 0).reshape(128, IT * B)
        ).astype(bf)
        in_maps.append(dict(base, x_t=xt))
    return in_maps


def run_device(inputs, n_steps=T, trace=False):
    from concourse.bass_utils import run_bass_kernel_spmd

    nc = _get_nc(n_steps)
    in_maps = _marshal(inputs)
    return run_bass_kernel_spmd(nc, in_maps, core_ids=list(range(N_CORES)),
                                trace=trace)


def combine(results):
    """Host-side unshard + tiny final statistics."""
    accs, steps_l, rem_l, curves, tmaxes = [], [], [], [], []
    nat_s = forc_s = 0.0
    for ci in range(N_CORES):
        r = results[ci]
        accs.append(np.ascontiguousarray(r["acc_t"].T))
        stats = r["stats"]
        steps_l.append(stats[0])
        rem_l.append(stats[1])
        nat_s += float(stats[2].sum(dtype=np.float64))
        forc_s += float(stats[3].sum(dtype=np.float64))
        curves.append(r["curve"][0, :T].copy())
        tmaxes.append(int(stats[0].max()))

    acc_state = np.concatenate(accs, axis=0)
    steps = np.concatenate(steps_l)
    rem_v = np.concatenate(rem_l)

    # pad each core's curve past its last executed step with its final value
    curve = np.zeros(T, dtype=np.float64)
    for ci in range(N_CORES):
        c = curves[ci].astype(np.float64)
        tm = max(tmaxes[ci], 1)
        c[tm:] = c[tm - 1]
        curve += c
    curve = (curve / B_FULL).astype(np.float32)

    ponder = (steps + rem_v) * TIME_PENALTY
    final_ponder = np.float32(ponder.mean(dtype=np.float64))
    remainder_mean = np.float32(rem_v.mean(dtype=np.float64))
    remainder_std = np.float32(rem_v.std(dtype=np.float64))
    natural_ratio = np.float32(nat_s / B_FULL)
    forced_ratio = np.float32(forc_s / B_FULL)
    p50 = np.float32(np.quantile(steps.astype(np.float64), 0.5))
    p90 = np.float32(np.quantile(steps.astype(np.float64), 0.9))
    return (acc_state, final_ponder, steps, remainder_mean, remainder_std,
            natural_ratio, forced_ratio, p50, p90, curve)


def kernel(**inputs):
    res = run_device(inputs, n_steps=T, trace=False)
    return combine(res.results)


if __name__ == "__main__":
    import os
    import time

    t0 = time.time()
    nc, nfix = _build(int(os.environ.get("NSTEPS", T)))
    print(f"built ok in {time.time() - t0:.1f}s, waitsplit fixes: {nfix}")


# revision 22
# speedup vs baseline: 6.2608x; 1.0883x over previous
"""AdaptiveRNNCell (ACT-halting GRU) Trainium2 kernel, 8-core data-parallel.

B=8192 batch sharded 1024/core; GRU weights replicated. All per-step state is
kept transposed [H-on-partitions, batch-on-free] so the recurrent matmul
h @ W_hh^T needs no per-step transposes. Weights are shipped pre-transposed /
tile-blocked in bf16 (host-side input marshaling), so the device does only
the input projection and the recurrent steps. Halting is per-sample; steps
1..19 are wrapped in runtime If(any_still_running) so the kernel stops
computing once every sample has halted (the torch module breaks early; with
halt bias 1.0 nearly everything halts after ~2 steps). Final scalar
statistics (means / quantiles / curve padding) are reduced on host from tiny
per-core vectors.
"""

import sys

for _p in ("/root/.axon_site/_ro/trn_rl_repo", "/opt/trn_rl_repo"):
    if _p not in sys.path:
        sys.path.append(_p)

import ml_dtypes
import numpy as np

import concourse.bass as bass
import concourse.mybir as mybir
import concourse.tile as tile
from concourse.masks import make_identity

N_CORES = 8
B_FULL, IN, H = 8192, 1024, 2048
B = B_FULL // N_CORES  # 1024 per core
G3 = 3 * H  # 6144
KT = H // 128  # 16 h tiles
GT = G3 // 128  # 48 gate tiles
IT = IN // 128  # 8 input tiles
CH = 512  # matmul moving chunk
NCH = B // CH  # 2
T = 20
THRESH = np.float32(1.0 - 0.01)
TIME_PENALTY = np.float32(0.001)

F32 = mybir.dt.float32
BF16 = mybir.dt.bfloat16
I32 = mybir.dt.int32
AF = mybir.ActivationFunctionType
ALU = mybir.AluOpType
AX = mybir.AxisListType


# ---------------------------------------------------------------- shims ----
def _patch_tile_drain():
    """walrus here rejects >1 sem wait on CTRL instructions: split the tile
    kernel-tail drain's waits across single-wait NOPs."""
    if getattr(tile.TileContext, "_drain_patched", False):
        return
    from concourse.vector_clock import ScopedClock

    def _patched(self, tick_clock, wait_clock):
        nc = self.nc
        drain_inst = nc.sync.drain()
        wait_clock.add_sem_waits(
            drain_inst.ins, ScopedClock({None: tick_clock.global_clock})
        )
        waits = list(drain_inst.ins.sync_info.on_wait)
        if len(waits) > 1:
            drain_inst.ins.sync_info = mybir.SyncInfo(
                on_wait=waits[:1], on_update=[]
            )
            for w in waits[1:]:
                nop = nc.sync.nop(nofuse=True)
                nop.ins.sync_info = mybir.SyncInfo(on_wait=[w], on_update=[])
        nc.all_engine_barrier()
        assert self.sems is not None
        popped = nc._tile_sem_poison_stack.pop()
        assert popped is self._sem_poison
        nc.clear_and_free_semaphores(list(self.sems.allocated().values()))
        nc.all_engine_barrier()

    tile.TileContext._drain_and_barrier = _patched
    tile.TileContext._drain_patched = True


def _split_excess_waits(nc, limit=1, max_upd=63):
    """walrus here caps sem waits per instruction and only supports small
    sem increments on compute instructions. Hoist excess waits onto
    same-engine NOPs and oversized sem-add updates onto EventSemaphore
    instructions emitted right after the owner."""
    n_fixed = 0
    for fn in nc.m.functions:
        for blk in fn.blocks:
            changed = False
            new_list = []
            for inst in blk.instructions:
                si = inst.sync_info
                waits = list(si.on_wait) if si is not None else []
                upds = list(si.on_update) if si is not None else []
                big_upds = [
                    u for u in upds
                    if getattr(u, "update_mode", "") == "sem-add-imm"
                    and getattr(u, "update_value", 0) > 1
                    and inst.opcode not in ("EventSemaphore", "ISA",
                                            "DMACopy", "Drain", "NoOp")
                ]
                if len(waits) > limit or big_upds:
                    hoist, keep = waits[:-limit], waits[-limit:]
                    for w in hoist:
                        n_fixed += 1
                        nop = mybir.InstNoOp(
                            name=f"waitsplit-{n_fixed}-{inst.name}", ins=[], outs=[]
                        )
                        nop.engine = inst.engine
                        nop.sync_info = mybir.SyncInfo(on_wait=[w], on_update=[])
                        new_list.append(nop)
                    keep_upds = [u for u in upds if u not in big_upds]
                    tail = []
                    for u in big_upds:
                        left = u.update_value - 1
                        ku = mybir.SyncUpdate(
                            ant_name=u.ant_name, id=u.id,
                            sync_type=u.sync_type,
                            update_mode="sem-inc", update_value=1)
                        keep_upds.append(ku)
                        while left > 0:
                            n_fixed += 1
                            cu = mybir.SyncUpdate(
                                ant_name=u.ant_name, id=u.id,
                                sync_type=u.sync_type,
                                update_mode="sem-add-imm",
                                update_value=min(left, max_upd))
                            ev = mybir.InstEventSemaphore(
                                name=f"updsplit-{n_fixed}-{inst.name}",
                                ins=[], outs=[])
                            ev.engine = inst.engine
                            ev.sync_info = mybir.SyncInfo(on_wait=[],
                                                          on_update=[cu])
                            tail.append(ev)
                            left -= max_upd
                    inst.sync_info = mybir.SyncInfo(
                        on_wait=keep, on_update=keep_upds
                    )
                    new_list.append(inst)
                    new_list.extend(tail)
                    changed = True
                else:
                    new_list.append(inst)
            if changed:
                blk.instructions = new_list
    return n_fixed


# ------------------------------------------------------------- builder ----
def _build(n_steps=T):
    _patch_tile_drain()
    nc = bass.Bass("TRN2", target_bir_lowering=False, debug=False,
                   num_devices=N_CORES)

    # host-marshaled inputs (pre-transposed / tile-blocked / pre-cast)
    xt_e = nc.dram_tensor("x_t", [128, IT * B], BF16, kind="ExternalInput")
    wih_e = nc.dram_tensor("w_ih_t", [GT, 128, IT * 128], BF16,
                           kind="ExternalInput")
    whh_e = nc.dram_tensor("w_hh_t", [GT, 128, KT * 128], BF16,
                           kind="ExternalInput")
    bih_e = nc.dram_tensor("bih_p", [128, GT], F32, kind="ExternalInput")
    bhh_e = nc.dram_tensor("bhh_p", [128, GT], F32, kind="ExternalInput")
    wf_e = nc.dram_tensor("wf_p", [128, GT], F32, kind="ExternalInput")
    hw_e = nc.dram_tensor("hw_p", [128, KT], BF16, kind="ExternalInput")
    hb_e = nc.dram_tensor("hb_p", [1, 1], F32, kind="ExternalInput")

    acc_e = nc.dram_tensor("acc_t", [H, B], F32, kind="ExternalOutput")
    stats_e = nc.dram_tensor("stats", [4, B], F32, kind="ExternalOutput")
    curve_e = nc.dram_tensor("curve", [1, 32], F32, kind="ExternalOutput")

    with tile.TileContext(nc) as tc:
        _body(nc, tc, n_steps, xt_e, wih_e, whh_e, bih_e, bhh_e, wf_e,
              hw_e, hb_e, acc_e, stats_e, curve_e)

    nfix = _split_excess_waits(nc, limit=1)
    return nc, nfix


def _body(nc, tc, n_steps, xt_e, wih_e, whh_e, bih_e, bhh_e, wf_e,
          hw_e, hb_e, acc_e, stats_e, curve_e):
    from contextlib import ExitStack

    with ExitStack() as st:
        cpool = st.enter_context(tc.tile_pool(name="const", bufs=1))
        wpool = st.enter_context(tc.tile_pool(name="wstream", bufs=5))
        xwpool = st.enter_context(tc.tile_pool(name="xwstream", bufs=4))
        gpool = st.enter_context(tc.tile_pool(name="gates", bufs=2))
        pspool = st.enter_context(tc.tile_pool(name="ps", bufs=6, space="PSUM"))
        pppool = st.enter_context(tc.tile_pool(name="pp", bufs=2, space="PSUM"))
        dpool = st.enter_context(tc.tile_pool(name="dram", bufs=1, space="DRAM"))

        # ---- constants -----------------------------------------------
        ident_b = cpool.tile([128, 128], BF16, tag="identb")
        make_identity(nc, ident_b[:])
        ones_col = cpool.tile([1, 128], F32, tag="ones")
        nc.gpsimd.memset(ones_col[:], 1.0)

        halt_w_sb = cpool.tile([128, KT], BF16, tag="haltw")
        nc.sync.dma_start(halt_w_sb[:], hw_e.ap())
        hb_sb = cpool.tile([1, 1], F32, tag="haltb")
        nc.sync.dma_start(hb_sb[:], hb_e.ap())

        bih_sb = cpool.tile([128, GT], F32, tag="bih")
        bhh_sb = cpool.tile([128, GT], F32, tag="bhh")
        wf_sb = cpool.tile([128, GT], F32, tag="wf")
        nc.sync.dma_start(bih_sb[:], bih_e.ap())
        nc.sync.dma_start(bhh_sb[:], bhh_e.ap())
        nc.sync.dma_start(wf_sb[:], wf_e.ap())
        bias_sum = cpool.tile([128, GT], F32, tag="bsum")  # b_ih + b_hh
        nc.vector.tensor_tensor(bias_sum[:], bih_sb[:], bhh_sb[:], ALU.add)
        bias0_sum = cpool.tile([128, GT], F32, tag="b0sum")  # + w_flag (step 0)
        nc.vector.tensor_tensor(bias0_sum[:], bias_sum[:], wf_sb[:], ALU.add)
        bias0_n = cpool.tile([128, GT], F32, tag="b0n")  # b_ih + w_flag (step 0)
        nc.vector.tensor_tensor(bias0_n[:], bih_sb[:], wf_sb[:], ALU.add)

        # ---- persistent per-sample rows ------------------------------
        def row(tag, init=0.0):
            t = cpool.tile([1, B], F32, tag=tag, name=f"row_{tag}")
            nc.gpsimd.memset(t[:], init)
            return t

        still = row("still", 1.0)
        acc_p = row("accp")
        steps_r = row("steps")
        rem_v = row("remv")
        nat_r = row("nat")
        forc_r = row("forc")
        p_sb = row("prow")
        new_acc = row("newacc")
        one_m = row("onem")
        natf = row("natf")
        p_adj = row("padj")
        rem = row("rem")
        sel = row("sel")

        curve_sb = cpool.tile([1, 32], F32, tag="curve")
        nc.gpsimd.memset(curve_sb[:], 0.0)
        count_sb = cpool.tile([1, 1], F32, tag="count")
        nc.gpsimd.memset(count_sb[:], 1.0)

        # ---- DRAM scratch --------------------------------------------
        xw_ts = [dpool.tile([128, B], BF16, tag=f"xw{j}", name=f"xwts{j}")
                 for j in range(GT)]
        acc_ts = [[dpool.tile([128, CH], F32, tag=f"acc{i}_{c}",
                              name=f"accts{i}_{c}")
                   for c in range(NCH)] for i in range(KT)]

        hpool_b = st.enter_context(tc.tile_pool(name="hbufb", bufs=1))
        h_b = hpool_b.tile([128, KT * B], BF16, tag="hb")

        # ---- halting tail (shared by all steps) ----------------------
        def acc_pass(t, h_nxt, to_out=False):
            # acc_state accumulation: acc += p_adj * h_nxt  (HWDGE RMW).
            # to_out: this step is the last executed one - write straight to
            # the external output instead of the scratch accumulator.
            for c in range(NCH):
                pP = pspool.tile([128, CH], F32, tag="ps")
                nc.tensor.matmul(pP[:], ones_col[:],
                                 p_adj[0:1, c * CH:(c + 1) * CH],
                                 start=True, stop=True)
                for i in range(KT):
                    dst = (acc_e.ap()[i * 128:(i + 1) * 128,
                                      c * CH:(c + 1) * CH]
                           if to_out else acc_ts[i][c][:])
                    tmp = gpool.tile([128, CH], F32, tag="acctmp", bufs=4)
                    nc.vector.tensor_tensor(
                        tmp[:], pP[:],
                        h_nxt[:, i * B + c * CH: i * B + c * CH + CH],
                        ALU.mult)
                    if t == 0:
                        nc.sync.dma_start(dst, tmp[:])
                    else:
                        a_in = gpool.tile([128, CH], F32, tag="accin", bufs=4)
                        nc.sync.dma_start(a_in[:], acc_ts[i][c][:])
                        a_new = gpool.tile([128, CH], F32, tag="accnew",
                                           bufs=4)
                        eng = nc.vector if i % 2 == 0 else nc.gpsimd
                        eng.tensor_tensor(a_new[:], a_in[:], tmp[:], ALU.add)
                        nc.sync.dma_start(dst, a_new[:])
            # consume-once: stale re-runs (skipped-step Else paths) add zero
            nc.gpsimd.memset(p_adj[:], 0.0)

        def halt_tail(t, h_nxt, ps_p, defer_acc=False):
            for c in range(NCH):
                nc.scalar.activation(p_sb[0:1, c * CH:(c + 1) * CH],
                                     ps_p[c][:], AF.Sigmoid, bias=hb_sb[:])
            nc.vector.tensor_tensor(new_acc[:], acc_p[:], p_sb[:], ALU.add)
            nc.gpsimd.tensor_scalar(one_m[:], acc_p[:], -1.0, 1.0,
                                    ALU.mult, ALU.add)
            nc.vector.tensor_scalar(natf[:], new_acc[:], float(THRESH),
                                    None, ALU.is_ge)
            nc.vector.tensor_tensor(natf[:], natf[:], still[:], ALU.mult)
            if t == T - 1:
                # forced halt of everyone still running
                nc.vector.tensor_tensor(p_adj[:], still[:], one_m[:], ALU.mult)
                nc.vector.tensor_copy(rem[:], p_adj[:])
                nc.vector.tensor_tensor(sel[:], still[:], natf[:],
                                        ALU.subtract)
                nc.vector.tensor_tensor(forc_r[:], forc_r[:], sel[:], ALU.add)
            else:
                # where(natf, one_m, p) == p + natf * (one_m - p)
                nc.vector.tensor_tensor(sel[:], one_m[:], p_sb[:],
                                        ALU.subtract)
                nc.vector.tensor_tensor(sel[:], natf[:], sel[:], ALU.mult)
                nc.vector.tensor_tensor(sel[:], sel[:], p_sb[:], ALU.add)
                nc.vector.tensor_tensor(p_adj[:], sel[:], still[:], ALU.mult)
                nc.vector.tensor_tensor(rem[:], natf[:], one_m[:], ALU.mult)
            nc.vector.scalar_tensor_tensor(
                acc_p[:], acc_p[:], 0.0, p_adj[:], ALU.add, ALU.add,
                accum_out=curve_sb[0:1, t:t + 1],
            )
            nc.gpsimd.tensor_tensor(steps_r[:], steps_r[:], still[:], ALU.add)
            nc.gpsimd.tensor_tensor(rem_v[:], rem_v[:], rem[:], ALU.add)
            nc.gpsimd.tensor_tensor(nat_r[:], nat_r[:], natf[:], ALU.add)
            if t == T - 1:
                nc.gpsimd.memset(still[:], 0.0)
            else:
                nc.vector.tensor_tensor(still[:], still[:], natf[:],
                                        ALU.subtract)
                nc.vector.tensor_reduce(count_sb[:], still[:], AX.X, ALU.add)
            if not defer_acc:
                acc_pass(t, h_nxt)

        # ---- fused input projection + step 0 -------------------------
        # h(0)=0, so step-0 gates are activations of the projection psum
        # directly; the psum is also copied out to xw_ts for later steps.
        h_nxt = h_b
        ps_p0 = [pppool.tile([1, CH], F32, tag="pp", name=f"psp0_{c}")
                 for c in range(NCH)]
        with tc.tile_pool(name="xtp", bufs=1) as xtpool:
            xT = xtpool.tile([128, IT * B], BF16, tag="xT")
            nc.sync.dma_start(xT[:], xt_e.ap())
            for i in range(KT):
                jts = (i, KT + i, 2 * KT + i)
                wTs = []
                xsts = []
                for jt in jts:
                    wT = wpool.tile([128, IT * 128], BF16, tag="wblk",
                                    name=f"pwT{jt}")
                    nc.sync.dma_start(wT[:], wih_e.ap()[jt])
                    wTs.append(wT)
                    xsts.append(xwpool.tile([128, B], BF16, tag="xwst",
                                            name=f"pxst{jt}"))
                for c in range(NCH):
                    sl = slice(c * CH, (c + 1) * CH)
                    hsl = slice(i * B + c * CH, i * B + (c + 1) * CH)
                    pxs = []
                    for g in range(3):
                        px = pspool.tile([128, CH], F32, tag="ps")
                        for it in range(IT):
                            nc.tensor.matmul(
                                px[:],
                                wTs[g][:, it * 128:(it + 1) * 128],
                                xT[:, it * B + c * CH: it * B + c * CH + CH],
                                start=(it == 0), stop=(it == IT - 1),
                            )
                        nc.vector.tensor_copy(xsts[g][:, sl], px[:])
                        pxs.append(px)
                    r0 = gpool.tile([128, CH], BF16, tag="r")
                    nc.scalar.activation(r0[:], pxs[0][:], AF.Sigmoid,
                                         bias=bias0_sum[:, i:i + 1])
                    z0 = gpool.tile([128, CH], BF16, tag="z")
                    nc.scalar.activation(z0[:], pxs[1][:], AF.Sigmoid,
                                         bias=bias0_sum[:, KT + i:KT + i + 1])
                    u0 = gpool.tile([128, CH], BF16, tag="u")
                    nc.vector.scalar_tensor_tensor(
                        u0[:], r0[:], bhh_sb[:, 2 * KT + i:2 * KT + i + 1],
                        pxs[2][:], ALU.mult, ALU.add)
                    n0 = gpool.tile([128, CH], BF16, tag="n")
                    nc.scalar.activation(
                        n0[:], u0[:], AF.Tanh,
                        bias=bias0_n[:, 2 * KT + i:2 * KT + i + 1])
                    e0 = gpool.tile([128, CH], BF16, tag="e")
                    nc.vector.tensor_tensor(e0[:], z0[:], n0[:], ALU.mult)
                    nc.vector.tensor_tensor(h_nxt[:, hsl], n0[:], e0[:],
                                            ALU.subtract)
                    nc.tensor.matmul(ps_p0[c][:], halt_w_sb[:, i:i + 1],
                                     h_nxt[:, hsl],
                                     start=(i == 0), stop=(i == KT - 1))
                for g, jt in enumerate(jts):
                    nc.sync.dma_start(xw_ts[jt][:], xsts[g][:])

        hpool_a = st.enter_context(tc.tile_pool(name="hbufa", bufs=1))
        h_a = hpool_a.tile([128, KT * B], BF16, tag="ha")
        hbufs = [h_a, h_b]
        halt_tail(0, h_nxt, ps_p0, defer_acc=True)

        # ---- steps 1..n_steps-1 with early exit ----------------------
        def step_body(t):
            h_cur = hbufs[t % 2]
            h_nx = hbufs[(t + 1) % 2]
            ps_p_t = [pppool.tile([1, CH], F32, tag="pp", name=f"pspt{t}_{c}")
                      for c in range(NCH)]
            for i in range(KT):
                jr, jz, jn = i, KT + i, 2 * KT + i
                w_r = wpool.tile([128, KT * 128], BF16, tag="wblk")
                w_z = wpool.tile([128, KT * 128], BF16, tag="wblk")
                w_n = wpool.tile([128, KT * 128], BF16, tag="wblk")
                nc.sync.dma_start(w_r[:], whh_e.ap()[jr])
                nc.sync.dma_start(w_z[:], whh_e.ap()[jz])
                nc.sync.dma_start(w_n[:], whh_e.ap()[jn])
                xw_r = xwpool.tile([128, B], BF16, tag="xw")
                xw_z = xwpool.tile([128, B], BF16, tag="xw")
                xw_n = xwpool.tile([128, B], BF16, tag="xw")
                nc.sync.dma_start(xw_r[:], xw_ts[jr][:])
                nc.sync.dma_start(xw_z[:], xw_ts[jz][:])
                nc.sync.dma_start(xw_n[:], xw_ts[jn][:])
                for c in range(NCH):
                    sl = slice(c * CH, (c + 1) * CH)
                    hsl = slice(i * B + c * CH, i * B + (c + 1) * CH)
                    ps_r = pspool.tile([128, CH], F32, tag="ps")
                    ps_z = pspool.tile([128, CH], F32, tag="ps")
                    ps_n = pspool.tile([128, CH], F32, tag="ps")
                    for kt in range(KT):
                        rh = h_cur[:, kt * B + c * CH: kt * B + (c + 1) * CH]
                        nc.tensor.matmul(
                            ps_r[:], w_r[:, kt * 128:(kt + 1) * 128], rh,
                            start=(kt == 0), stop=False)
                        nc.tensor.matmul(
                            ps_z[:], w_z[:, kt * 128:(kt + 1) * 128], rh,
                            start=(kt == 0), stop=False)
                        nc.tensor.matmul(
                            ps_n[:], w_n[:, kt * 128:(kt + 1) * 128], rh,
                            start=(kt == 0), stop=(kt == KT - 1))
                    nc.tensor.matmul(ps_r[:], ident_b[:], xw_r[:, sl],
                                     start=False, stop=True)
                    nc.tensor.matmul(ps_z[:], ident_b[:], xw_z[:, sl],
                                     start=False, stop=True)
                    r_t = gpool.tile([128, CH], BF16, tag="r")
                    nc.scalar.activation(r_t[:], ps_r[:], AF.Sigmoid,
                                         bias=bias_sum[:, jr:jr + 1])
                    z_t = gpool.tile([128, CH], BF16, tag="z")
                    nc.scalar.activation(z_t[:], ps_z[:], AF.Sigmoid,
                                         bias=bias_sum[:, jz:jz + 1])
                    u_t = gpool.tile([128, CH], BF16, tag="u")
                    nc.vector.scalar_tensor_tensor(
                        u_t[:], ps_n[:], bhh_sb[:, jn:jn + 1], r_t[:],
                        ALU.add, ALU.mult)
                    v_t = gpool.tile([128, CH], BF16, tag="v")
                    nc.vector.tensor_tensor(v_t[:], u_t[:], xw_n[:, sl],
                                            ALU.add)
                    n_t = gpool.tile([128, CH], BF16, tag="n")
                    nc.scalar.activation(n_t[:], v_t[:], AF.Tanh,
                                         bias=bih_sb[:, jn:jn + 1])
                    d_t = gpool.tile([128, CH], BF16, tag="d")
                    nc.vector.tensor_tensor(d_t[:], h_cur[:, hsl], n_t[:],
                                            ALU.subtract)
                    e_t = gpool.tile([128, CH], BF16, tag="e")
                    nc.vector.tensor_tensor(e_t[:], z_t[:], d_t[:], ALU.mult)
                    nc.vector.tensor_tensor(h_nx[:, hsl], n_t[:], e_t[:],
                                            ALU.add)
                    nc.tensor.matmul(ps_p_t[c][:], halt_w_sb[:, i:i + 1],
                                     h_nx[:, hsl],
                                     start=(i == 0), stop=(i == KT - 1))
            halt_tail(t, h_nx, ps_p_t, defer_acc=(t == 1))

        def load_count():
            return nc.values_load(
                count_sb[:].bitcast(I32), min_val=0, max_val=2 ** 30,
                skip_runtime_bounds_check=True)

        def final_copies():
            for i in range(KT):
                for c in range(NCH):
                    nc.sync.dma_start(
                        acc_e.ap()[i * 128:(i + 1) * 128,
                                   c * CH:(c + 1) * CH],
                        acc_ts[i][c][:])

        if n_steps == 1:
            acc_pass(0, hbufs[1], to_out=True)
        else:
            c1 = load_count()
            with tc.If(c1 > 0) as cmp1:
                acc_pass(0, hbufs[1])
                step_body(1)
            with cmp1.Else():
                acc_pass(0, hbufs[1])
            if n_steps == 2:
                acc_pass(1, hbufs[0], to_out=True)
            else:
                c2 = load_count()
                with tc.If(c2 > 0) as cmpo:
                    acc_pass(1, hbufs[0])
                    for t in range(2, n_steps):
                        ct = load_count()
                        with tc.If(ct > 0):
                            step_body(t)
                    final_copies()
                with cmpo.Else():
                    acc_pass(1, hbufs[0], to_out=True)

        # ---- final outputs -------------------------------------------
        nc.sync.dma_start(stats_e.ap()[0:1, :], steps_r[:])
        nc.sync.dma_start(stats_e.ap()[1:2, :], rem_v[:])
        nc.sync.dma_start(stats_e.ap()[2:3, :], nat_r[:])
        nc.sync.dma_start(stats_e.ap()[3:4, :], forc_r[:])
        nc.sync.dma_start(curve_e.ap()[:, :], curve_sb[:])


# --------------------------------------------------------------- runner ----
_CACHE = {}


def _get_nc(n_steps=T):
    if n_steps not in _CACHE:
        _CACHE[n_steps] = _build(n_steps)[0]
    return _CACHE[n_steps]


def _marshal(inputs):
    """Host-side input marshaling: shard x, pre-transpose / tile-block /
    bf16-cast the replicated weights into the layouts the device consumes."""
    bf = ml_dtypes.bfloat16
    x = np.asarray(inputs["input_tensor"], dtype=np.float32)
    w_ih = np.asarray(inputs["weight_ih"], dtype=np.float32)
    w_hh = np.asarray(inputs["weight_hh"], dtype=np.float32)
    b_ih = np.asarray(inputs["bias_ih"], dtype=np.float32)
    b_hh = np.asarray(inputs["bias_hh"], dtype=np.float32)
    halt_w = np.asarray(inputs["halt_w"], dtype=np.float32)
    halt_b = np.asarray(inputs["halt_b"], dtype=np.float32)

    # w_ih_t[jt, p, it*128+j'] = w_ih[jt*128+j', it*128+p]
    wih_blk = w_ih[:, :IN].reshape(GT, 128, IT, 128)  # [jt, j', it, p]
    wih_blk = np.ascontiguousarray(
        wih_blk.transpose(0, 3, 2, 1).reshape(GT, 128, IT * 128)
    ).astype(bf)

    # w_hh_t[jt, p, kt*128+j'] = w_hh[jt*128+j', kt*128+p]
    whh_blk = w_hh.reshape(GT, 128, KT, 128)  # [jt, j', kt, p]
    whh_blk = np.ascontiguousarray(
        whh_blk.transpose(0, 3, 2, 1).reshape(GT, 128, KT * 128)
    ).astype(bf)

    bih_p = np.ascontiguousarray(b_ih.reshape(GT, 128).T)
    bhh_p = np.ascontiguousarray(b_hh.reshape(GT, 128).T)
    wf_p = np.ascontiguousarray(w_ih[:, IN].reshape(GT, 128).T)
    hw_p = np.ascontiguousarray(halt_w.reshape(KT, 128).T.astype(bf))
    hb_p = np.ascontiguousarray(halt_b.reshape(1, 1))

    base = {
        "w_ih_t": wih_blk, "w_hh_t": whh_blk,
        "bih_p": bih_p, "bhh_p": bhh_p, "wf_p": wf_p,
        "hw_p": hw_p, "hb_p": hb_p,
    }
    in_maps = []
    for ci in range(N_CORES):
        xs = x[ci * B:(ci + 1) * B]  # [B, IN]
        # x_t[p, it*B + b] = x[b, it*128+p]
        xt = np.ascontiguousarray(
            xs.reshape(B, IT, 128).transpose(2, 1, 0).reshape(128, IT * B)
        ).astype(bf)
        in_maps.append(dict(base, x_t=xt))
    return in_maps


def run_device(inputs, n_steps=T, trace=False):
    from concourse.bass_utils import run_bass_kernel_spmd

    nc = _get_nc(n_steps)
    in_maps = _marshal(inputs)
    return run_bass_kernel_spmd(nc, in_maps, core_ids=list(range(N_CORES)),
                                trace=trace)


def combine(results):
    """Host-side unshard + tiny final statistics."""
    accs, steps_l, rem_l, curves, tmaxes = [], [], [], [], []
    nat_s = forc_s = 0.0
    for ci in range(N_CORES):
        r = results[ci]
        accs.append(np.ascontiguousarray(r["acc_t"].T))
        stats = r["stats"]
        steps_l.append(stats[0])
        rem_l.append(stats[1])
        nat_s += float(stats[2].sum(dtype=np.float64))
        forc_s += float(stats[3].sum(dtype=np.float64))
        curves.append(r["curve"][0, :T].copy())
        tmaxes.append(int(stats[0].max()))

    acc_state = np.concatenate(accs, axis=0)
    steps = np.concatenate(steps_l)
    rem_v = np.concatenate(rem_l)

    # pad each core's curve past its last executed step with its final value
    curve = np.zeros(T, dtype=np.float64)
    for ci in range(N_CORES):
        c = curves[ci].astype(np.float64)
        tm = max(tmaxes[ci], 1)
        c[tm:] = c[tm - 1]
        curve += c
    curve = (curve / B_FULL).astype(np.float32)

    ponder = (steps + rem_v) * TIME_PENALTY
    final_ponder = np.float32(ponder.mean(dtype=np.float64))
    remainder_mean = np.float32(rem_v.mean(dtype=np.float64))
    remainder_std = np.float32(rem_v.std(dtype=np.float64))
    natural_ratio = np.float32(nat_s / B_FULL)
    forced_ratio = np.float32(forc_s / B_FULL)
    p50 = np.float32(np.quantile(steps.astype(np.float64), 0.5))
    p90 = np.float32(np.quantile(steps.astype(np.float64), 0.9))
    return (acc_state, final_ponder, steps, remainder_mean, remainder_std,
            natural_ratio, forced_ratio, p50, p90, curve)


def kernel(**inputs):
    res = run_device(inputs, n_steps=T, trace=False)
    return combine(res.results)


if __name__ == "__main__":
    import os
    import time

    t0 = time.time()
    nc, nfix = _build(int(os.environ.get("NSTEPS", T)))
    print(f"built ok in {time.time() - t0:.1f}s, waitsplit fixes: {nfix}")


# revision 23
# speedup vs baseline: 6.4090x; 1.0237x over previous
"""AdaptiveRNNCell (ACT-halting GRU) Trainium2 kernel, 8-core data-parallel.

B=8192 batch sharded 1024/core; GRU weights replicated. All per-step state is
kept transposed [H-on-partitions, batch-on-free] so the recurrent matmul
h @ W_hh^T needs no per-step transposes. Weights are shipped pre-transposed /
tile-blocked in bf16 (host-side input marshaling), so the device does only
the input projection and the recurrent steps. Halting is per-sample; steps
1..19 are wrapped in runtime If(any_still_running) so the kernel stops
computing once every sample has halted (the torch module breaks early; with
halt bias 1.0 nearly everything halts after ~2 steps). Final scalar
statistics (means / quantiles / curve padding) are reduced on host from tiny
per-core vectors.
"""

import sys

for _p in ("/root/.axon_site/_ro/trn_rl_repo", "/opt/trn_rl_repo"):
    if _p not in sys.path:
        sys.path.append(_p)

import ml_dtypes
import numpy as np

import concourse.bass as bass
import concourse.mybir as mybir
import concourse.tile as tile
from concourse.masks import make_identity

N_CORES = 8
B_FULL, IN, H = 8192, 1024, 2048
B = B_FULL // N_CORES  # 1024 per core
G3 = 3 * H  # 6144
KT = H // 128  # 16 h tiles
GT = G3 // 128  # 48 gate tiles
IT = IN // 128  # 8 input tiles
CH = 512  # matmul moving chunk
NCH = B // CH  # 2
T = 20
THRESH = np.float32(1.0 - 0.01)
TIME_PENALTY = np.float32(0.001)

F32 = mybir.dt.float32
BF16 = mybir.dt.bfloat16
I32 = mybir.dt.int32
AF = mybir.ActivationFunctionType
ALU = mybir.AluOpType
AX = mybir.AxisListType


# ---------------------------------------------------------------- shims ----
def _patch_tile_drain():
    """walrus here rejects >1 sem wait on CTRL instructions: split the tile
    kernel-tail drain's waits across single-wait NOPs."""
    if getattr(tile.TileContext, "_drain_patched", False):
        return
    from concourse.vector_clock import ScopedClock

    def _patched(self, tick_clock, wait_clock):
        nc = self.nc
        drain_inst = nc.sync.drain()
        wait_clock.add_sem_waits(
            drain_inst.ins, ScopedClock({None: tick_clock.global_clock})
        )
        waits = list(drain_inst.ins.sync_info.on_wait)
        if len(waits) > 1:
            drain_inst.ins.sync_info = mybir.SyncInfo(
                on_wait=waits[:1], on_update=[]
            )
            for w in waits[1:]:
                nop = nc.sync.nop(nofuse=True)
                nop.ins.sync_info = mybir.SyncInfo(on_wait=[w], on_update=[])
        nc.all_engine_barrier()
        assert self.sems is not None
        popped = nc._tile_sem_poison_stack.pop()
        assert popped is self._sem_poison
        nc.clear_and_free_semaphores(list(self.sems.allocated().values()))
        nc.all_engine_barrier()

    tile.TileContext._drain_and_barrier = _patched
    tile.TileContext._drain_patched = True


def _split_excess_waits(nc, limit=1, max_upd=63):
    """walrus here caps sem waits per instruction and only supports small
    sem increments on compute instructions. Hoist excess waits onto
    same-engine NOPs and oversized sem-add updates onto EventSemaphore
    instructions emitted right after the owner."""
    n_fixed = 0
    for fn in nc.m.functions:
        for blk in fn.blocks:
            changed = False
            new_list = []
            for inst in blk.instructions:
                si = inst.sync_info
                waits = list(si.on_wait) if si is not None else []
                upds = list(si.on_update) if si is not None else []
                big_upds = [
                    u for u in upds
                    if getattr(u, "update_mode", "") == "sem-add-imm"
                    and getattr(u, "update_value", 0) > 1
                    and inst.opcode not in ("EventSemaphore", "ISA",
                                            "DMACopy", "Drain", "NoOp")
                ]
                if len(waits) > limit or big_upds:
                    hoist, keep = waits[:-limit], waits[-limit:]
                    for w in hoist:
                        n_fixed += 1
                        nop = mybir.InstNoOp(
                            name=f"waitsplit-{n_fixed}-{inst.name}", ins=[], outs=[]
                        )
                        nop.engine = inst.engine
                        nop.sync_info = mybir.SyncInfo(on_wait=[w], on_update=[])
                        new_list.append(nop)
                    keep_upds = [u for u in upds if u not in big_upds]
                    tail = []
                    for u in big_upds:
                        left = u.update_value - 1
                        ku = mybir.SyncUpdate(
                            ant_name=u.ant_name, id=u.id,
                            sync_type=u.sync_type,
                            update_mode="sem-inc", update_value=1)
                        keep_upds.append(ku)
                        while left > 0:
                            n_fixed += 1
                            cu = mybir.SyncUpdate(
                                ant_name=u.ant_name, id=u.id,
                                sync_type=u.sync_type,
                                update_mode="sem-add-imm",
                                update_value=min(left, max_upd))
                            ev = mybir.InstEventSemaphore(
                                name=f"updsplit-{n_fixed}-{inst.name}",
                                ins=[], outs=[])
                            ev.engine = inst.engine
                            ev.sync_info = mybir.SyncInfo(on_wait=[],
                                                          on_update=[cu])
                            tail.append(ev)
                            left -= max_upd
                    inst.sync_info = mybir.SyncInfo(
                        on_wait=keep, on_update=keep_upds
                    )
                    new_list.append(inst)
                    new_list.extend(tail)
                    changed = True
                else:
                    new_list.append(inst)
            if changed:
                blk.instructions = new_list
    return n_fixed


# ------------------------------------------------------------- builder ----
def _build(n_steps=T):
    _patch_tile_drain()
    nc = bass.Bass("TRN2", target_bir_lowering=False, debug=False,
                   num_devices=N_CORES)

    # host-marshaled inputs (pre-transposed / tile-blocked / pre-cast)
    xt_e = nc.dram_tensor("x_t", [128, IT * B], BF16, kind="ExternalInput")
    wih_e = nc.dram_tensor("w_ih_t", [GT, 128, IT * 128], BF16,
                           kind="ExternalInput")
    whh_e = nc.dram_tensor("w_hh_t", [GT, 128, KT * 128], BF16,
                           kind="ExternalInput")
    bih_e = nc.dram_tensor("bih_p", [128, GT], F32, kind="ExternalInput")
    bhh_e = nc.dram_tensor("bhh_p", [128, GT], F32, kind="ExternalInput")
    wf_e = nc.dram_tensor("wf_p", [128, GT], F32, kind="ExternalInput")
    hw_e = nc.dram_tensor("hw_p", [128, KT], BF16, kind="ExternalInput")
    hb_e = nc.dram_tensor("hb_p", [1, 1], F32, kind="ExternalInput")

    acc_e = nc.dram_tensor("acc_t", [H, B], F32, kind="ExternalOutput")
    stats_e = nc.dram_tensor("stats", [4, B], F32, kind="ExternalOutput")
    curve_e = nc.dram_tensor("curve", [1, 32], F32, kind="ExternalOutput")

    with tile.TileContext(nc) as tc:
        _body(nc, tc, n_steps, xt_e, wih_e, whh_e, bih_e, bhh_e, wf_e,
              hw_e, hb_e, acc_e, stats_e, curve_e)

    nfix = _split_excess_waits(nc, limit=1)
    return nc, nfix


def _body(nc, tc, n_steps, xt_e, wih_e, whh_e, bih_e, bhh_e, wf_e,
          hw_e, hb_e, acc_e, stats_e, curve_e):
    from contextlib import ExitStack

    with ExitStack() as st:
        cpool = st.enter_context(tc.tile_pool(name="const", bufs=1))
        wpool = st.enter_context(tc.tile_pool(name="wstream", bufs=5))
        xwpool = st.enter_context(tc.tile_pool(name="xwstream", bufs=4))
        gpool = st.enter_context(tc.tile_pool(name="gates", bufs=2))
        pspool = st.enter_context(tc.tile_pool(name="ps", bufs=6, space="PSUM"))
        pppool = st.enter_context(tc.tile_pool(name="pp", bufs=2, space="PSUM"))
        dpool = st.enter_context(tc.tile_pool(name="dram", bufs=1, space="DRAM"))

        # ---- constants -----------------------------------------------
        ident_b = cpool.tile([128, 128], BF16, tag="identb")
        make_identity(nc, ident_b[:])
        ones_col = cpool.tile([1, 128], F32, tag="ones")
        nc.gpsimd.memset(ones_col[:], 1.0)

        halt_w_sb = cpool.tile([128, KT], BF16, tag="haltw")
        nc.sync.dma_start(halt_w_sb[:], hw_e.ap())
        hb_sb = cpool.tile([1, 1], F32, tag="haltb")
        nc.sync.dma_start(hb_sb[:], hb_e.ap())

        bih_sb = cpool.tile([128, GT], F32, tag="bih")
        bhh_sb = cpool.tile([128, GT], F32, tag="bhh")
        wf_sb = cpool.tile([128, GT], F32, tag="wf")
        nc.sync.dma_start(bih_sb[:], bih_e.ap())
        nc.sync.dma_start(bhh_sb[:], bhh_e.ap())
        nc.sync.dma_start(wf_sb[:], wf_e.ap())
        bias_sum = cpool.tile([128, GT], F32, tag="bsum")  # b_ih + b_hh
        nc.vector.tensor_tensor(bias_sum[:], bih_sb[:], bhh_sb[:], ALU.add)
        bias0_sum = cpool.tile([128, GT], F32, tag="b0sum")  # + w_flag (step 0)
        nc.vector.tensor_tensor(bias0_sum[:], bias_sum[:], wf_sb[:], ALU.add)
        bias0_n = cpool.tile([128, GT], F32, tag="b0n")  # b_ih + w_flag (step 0)
        nc.vector.tensor_tensor(bias0_n[:], bih_sb[:], wf_sb[:], ALU.add)

        # ---- persistent per-sample rows ------------------------------
        def row(tag, init=0.0):
            t = cpool.tile([1, B], F32, tag=tag, name=f"row_{tag}")
            nc.gpsimd.memset(t[:], init)
            return t

        still = row("still", 1.0)
        acc_p = row("accp")
        steps_r = row("steps")
        rem_v = row("remv")
        nat_r = row("nat")
        forc_r = row("forc")
        p_sb = row("prow")
        new_acc = row("newacc")
        one_m = row("onem")
        natf = row("natf")
        p_adj = row("padj")
        rem = row("rem")
        sel = new_acc  # lifetimes disjoint: new_acc dead once natf computed

        curve_sb = cpool.tile([1, 32], F32, tag="curve")
        nc.gpsimd.memset(curve_sb[:], 0.0)
        count_sb = cpool.tile([1, 1], F32, tag="count")
        nc.gpsimd.memset(count_sb[:], 1.0)

        # ---- DRAM scratch --------------------------------------------
        xw_ts = [dpool.tile([128, B], BF16, tag=f"xw{j}", name=f"xwts{j}")
                 for j in range(GT)]
        acc_ts = [[dpool.tile([128, CH], F32, tag=f"acc{i}_{c}",
                              name=f"accts{i}_{c}")
                   for c in range(NCH)] for i in range(KT)]

        hpool_b = st.enter_context(tc.tile_pool(name="hbufb", bufs=1))
        h_b = hpool_b.tile([128, KT * B], BF16, tag="hb")

        # ---- halting tail (shared by all steps) ----------------------
        def acc_pass(t, h_nxt, to_out=False):
            # acc_state accumulation: acc += p_adj * h_nxt  (HWDGE RMW).
            # to_out: this step is the last executed one - write straight to
            # the external output instead of the scratch accumulator.
            for c in range(NCH):
                pP = pspool.tile([128, CH], F32, tag="ps")
                nc.tensor.matmul(pP[:], ones_col[:],
                                 p_adj[0:1, c * CH:(c + 1) * CH],
                                 start=True, stop=True)
                for i in range(KT):
                    dst = (acc_e.ap()[i * 128:(i + 1) * 128,
                                      c * CH:(c + 1) * CH]
                           if to_out else acc_ts[i][c][:])
                    tmp = gpool.tile([128, CH], F32, tag="acctmp", bufs=4)
                    nc.vector.tensor_tensor(
                        tmp[:], pP[:],
                        h_nxt[:, i * B + c * CH: i * B + c * CH + CH],
                        ALU.mult)
                    if t == 0:
                        nc.sync.dma_start(dst, tmp[:])
                    else:
                        a_in = gpool.tile([128, CH], F32, tag="accin", bufs=8)
                        nc.sync.dma_start(a_in[:], acc_ts[i][c][:])
                        a_new = gpool.tile([128, CH], F32, tag="accnew",
                                           bufs=4)
                        eng = nc.vector if i % 2 == 0 else nc.gpsimd
                        eng.tensor_tensor(a_new[:], a_in[:], tmp[:], ALU.add)
                        nc.sync.dma_start(dst, a_new[:])
            # consume-once: stale re-runs (skipped-step Else paths) add zero
            nc.gpsimd.memset(p_adj[:], 0.0)

        def halt_tail(t, h_nxt, ps_p, defer_acc=False):
            for c in range(NCH):
                nc.scalar.activation(p_sb[0:1, c * CH:(c + 1) * CH],
                                     ps_p[c][:], AF.Sigmoid, bias=hb_sb[:])
            nc.vector.tensor_tensor(new_acc[:], acc_p[:], p_sb[:], ALU.add)
            nc.gpsimd.tensor_scalar(one_m[:], acc_p[:], -1.0, 1.0,
                                    ALU.mult, ALU.add)
            nc.vector.tensor_scalar(natf[:], new_acc[:], float(THRESH),
                                    None, ALU.is_ge)
            nc.vector.tensor_tensor(natf[:], natf[:], still[:], ALU.mult)
            if t == T - 1:
                # forced halt of everyone still running
                nc.vector.tensor_tensor(p_adj[:], still[:], one_m[:], ALU.mult)
                nc.vector.tensor_copy(rem[:], p_adj[:])
                nc.vector.tensor_tensor(sel[:], still[:], natf[:],
                                        ALU.subtract)
                nc.vector.tensor_tensor(forc_r[:], forc_r[:], sel[:], ALU.add)
            else:
                # where(natf, one_m, p) == p + natf * (one_m - p)
                nc.vector.tensor_tensor(sel[:], one_m[:], p_sb[:],
                                        ALU.subtract)
                nc.vector.tensor_tensor(sel[:], natf[:], sel[:], ALU.mult)
                nc.vector.tensor_tensor(sel[:], sel[:], p_sb[:], ALU.add)
                nc.vector.tensor_tensor(p_adj[:], sel[:], still[:], ALU.mult)
                nc.vector.tensor_tensor(rem[:], natf[:], one_m[:], ALU.mult)
            nc.vector.scalar_tensor_tensor(
                acc_p[:], acc_p[:], 0.0, p_adj[:], ALU.add, ALU.add,
                accum_out=curve_sb[0:1, t:t + 1],
            )
            nc.gpsimd.tensor_tensor(steps_r[:], steps_r[:], still[:], ALU.add)
            nc.gpsimd.tensor_tensor(rem_v[:], rem_v[:], rem[:], ALU.add)
            nc.gpsimd.tensor_tensor(nat_r[:], nat_r[:], natf[:], ALU.add)
            if t == T - 1:
                nc.gpsimd.memset(still[:], 0.0)
            else:
                nc.vector.tensor_tensor(still[:], still[:], natf[:],
                                        ALU.subtract)
                nc.vector.tensor_reduce(count_sb[:], still[:], AX.X, ALU.add)
            if not defer_acc:
                acc_pass(t, h_nxt)

        # ---- fused input projection + step 0 -------------------------
        # h(0)=0, so step-0 gates are activations of the projection psum
        # directly; the psum is also copied out to xw_ts for later steps.
        h_nxt = h_b
        ps_p0 = [pppool.tile([1, CH], F32, tag="pp", name=f"psp0_{c}")
                 for c in range(NCH)]
        with tc.tile_pool(name="xtp", bufs=1) as xtpool:
            xT = xtpool.tile([128, IT * B], BF16, tag="xT")
            nc.sync.dma_start(xT[:], xt_e.ap())
            for i in range(KT):
                jts = (i, KT + i, 2 * KT + i)
                wTs = []
                xsts = []
                for jt in jts:
                    wT = wpool.tile([128, IT * 128], BF16, tag="wblk",
                                    name=f"pwT{jt}")
                    nc.sync.dma_start(wT[:], wih_e.ap()[jt])
                    wTs.append(wT)
                    xsts.append(xwpool.tile([128, B], BF16, tag="xwst",
                                            name=f"pxst{jt}"))
                for c in range(NCH):
                    sl = slice(c * CH, (c + 1) * CH)
                    hsl = slice(i * B + c * CH, i * B + (c + 1) * CH)
                    pxs = []
                    for g in range(3):
                        px = pspool.tile([128, CH], F32, tag="ps")
                        for it in range(IT):
                            nc.tensor.matmul(
                                px[:],
                                wTs[g][:, it * 128:(it + 1) * 128],
                                xT[:, it * B + c * CH: it * B + c * CH + CH],
                                start=(it == 0), stop=(it == IT - 1),
                            )
                        nc.vector.tensor_copy(xsts[g][:, sl], px[:])
                        pxs.append(px)
                    r0 = gpool.tile([128, CH], BF16, tag="r")
                    nc.scalar.activation(r0[:], pxs[0][:], AF.Sigmoid,
                                         bias=bias0_sum[:, i:i + 1])
                    z0 = gpool.tile([128, CH], BF16, tag="z")
                    nc.scalar.activation(z0[:], pxs[1][:], AF.Sigmoid,
                                         bias=bias0_sum[:, KT + i:KT + i + 1])
                    u0 = gpool.tile([128, CH], BF16, tag="u")
                    nc.vector.scalar_tensor_tensor(
                        u0[:], r0[:], bhh_sb[:, 2 * KT + i:2 * KT + i + 1],
                        pxs[2][:], ALU.mult, ALU.add)
                    n0 = gpool.tile([128, CH], BF16, tag="n")
                    nc.scalar.activation(
                        n0[:], u0[:], AF.Tanh,
                        bias=bias0_n[:, 2 * KT + i:2 * KT + i + 1])
                    e0 = gpool.tile([128, CH], BF16, tag="e")
                    nc.vector.tensor_tensor(e0[:], z0[:], n0[:], ALU.mult)
                    nc.vector.tensor_tensor(h_nxt[:, hsl], n0[:], e0[:],
                                            ALU.subtract)
                    nc.tensor.matmul(ps_p0[c][:], halt_w_sb[:, i:i + 1],
                                     h_nxt[:, hsl],
                                     start=(i == 0), stop=(i == KT - 1))
                for g, jt in enumerate(jts):
                    nc.sync.dma_start(xw_ts[jt][:], xsts[g][:])

        hpool_a = st.enter_context(tc.tile_pool(name="hbufa", bufs=1))
        h_a = hpool_a.tile([128, KT * B], BF16, tag="ha")
        hbufs = [h_a, h_b]
        halt_tail(0, h_nxt, ps_p0, defer_acc=True)

        # ---- steps 1..n_steps-1 with early exit ----------------------
        def step_body(t):
            h_cur = hbufs[t % 2]
            h_nx = hbufs[(t + 1) % 2]
            ps_p_t = [pppool.tile([1, CH], F32, tag="pp", name=f"pspt{t}_{c}")
                      for c in range(NCH)]
            for i in range(KT):
                jr, jz, jn = i, KT + i, 2 * KT + i
                w_r = wpool.tile([128, KT * 128], BF16, tag="wblk")
                w_z = wpool.tile([128, KT * 128], BF16, tag="wblk")
                w_n = wpool.tile([128, KT * 128], BF16, tag="wblk")
                nc.sync.dma_start(w_r[:], whh_e.ap()[jr])
                nc.sync.dma_start(w_z[:], whh_e.ap()[jz])
                nc.sync.dma_start(w_n[:], whh_e.ap()[jn])
                xw_r = xwpool.tile([128, B], BF16, tag="xw")
                xw_z = xwpool.tile([128, B], BF16, tag="xw")
                xw_n = xwpool.tile([128, B], BF16, tag="xw")
                nc.sync.dma_start(xw_r[:], xw_ts[jr][:])
                nc.sync.dma_start(xw_z[:], xw_ts[jz][:])
                nc.sync.dma_start(xw_n[:], xw_ts[jn][:])
                for c in range(NCH):
                    sl = slice(c * CH, (c + 1) * CH)
                    hsl = slice(i * B + c * CH, i * B + (c + 1) * CH)
                    ps_r = pspool.tile([128, CH], F32, tag="ps")
                    ps_z = pspool.tile([128, CH], F32, tag="ps")
                    ps_n = pspool.tile([128, CH], F32, tag="ps")
                    for kt in range(KT):
                        rh = h_cur[:, kt * B + c * CH: kt * B + (c + 1) * CH]
                        nc.tensor.matmul(
                            ps_r[:], w_r[:, kt * 128:(kt + 1) * 128], rh,
                            start=(kt == 0), stop=False)
                        nc.tensor.matmul(
                            ps_z[:], w_z[:, kt * 128:(kt + 1) * 128], rh,
                            start=(kt == 0), stop=False)
                        nc.tensor.matmul(
                            ps_n[:], w_n[:, kt * 128:(kt + 1) * 128], rh,
                            start=(kt == 0), stop=(kt == KT - 1))
                    nc.tensor.matmul(ps_r[:], ident_b[:], xw_r[:, sl],
                                     start=False, stop=True)
                    nc.tensor.matmul(ps_z[:], ident_b[:], xw_z[:, sl],
                                     start=False, stop=True)
                    r_t = gpool.tile([128, CH], BF16, tag="r")
                    nc.scalar.activation(r_t[:], ps_r[:], AF.Sigmoid,
                                         bias=bias_sum[:, jr:jr + 1])
                    z_t = gpool.tile([128, CH], BF16, tag="z")
                    nc.scalar.activation(z_t[:], ps_z[:], AF.Sigmoid,
                                         bias=bias_sum[:, jz:jz + 1])
                    u_t = gpool.tile([128, CH], BF16, tag="u")
                    nc.vector.scalar_tensor_tensor(
                        u_t[:], ps_n[:], bhh_sb[:, jn:jn + 1], r_t[:],
                        ALU.add, ALU.mult)
                    v_t = gpool.tile([128, CH], BF16, tag="v")
                    nc.vector.tensor_tensor(v_t[:], u_t[:], xw_n[:, sl],
                                            ALU.add)
                    n_t = gpool.tile([128, CH], BF16, tag="n")
                    nc.scalar.activation(n_t[:], v_t[:], AF.Tanh,
                                         bias=bih_sb[:, jn:jn + 1])
                    d_t = gpool.tile([128, CH], BF16, tag="d")
                    nc.vector.tensor_tensor(d_t[:], h_cur[:, hsl], n_t[:],
                                            ALU.subtract)
                    e_t = gpool.tile([128, CH], BF16, tag="e")
                    nc.vector.tensor_tensor(e_t[:], z_t[:], d_t[:], ALU.mult)
                    nc.vector.tensor_tensor(h_nx[:, hsl], n_t[:], e_t[:],
                                            ALU.add)
                    nc.tensor.matmul(ps_p_t[c][:], halt_w_sb[:, i:i + 1],
                                     h_nx[:, hsl],
                                     start=(i == 0), stop=(i == KT - 1))
            halt_tail(t, h_nx, ps_p_t, defer_acc=(t == 1))

        def load_count():
            return nc.values_load(
                count_sb[:].bitcast(I32), min_val=0, max_val=2 ** 30,
                skip_runtime_bounds_check=True)

        def final_copies():
            for i in range(KT):
                for c in range(NCH):
                    nc.sync.dma_start(
                        acc_e.ap()[i * 128:(i + 1) * 128,
                                   c * CH:(c + 1) * CH],
                        acc_ts[i][c][:])

        if n_steps == 1:
            acc_pass(0, hbufs[1], to_out=True)
        else:
            c1 = load_count()
            with tc.If(c1 > 0) as cmp1:
                acc_pass(0, hbufs[1])
                step_body(1)
            with cmp1.Else():
                acc_pass(0, hbufs[1])
            if n_steps == 2:
                acc_pass(1, hbufs[0], to_out=True)
            else:
                c2 = load_count()
                with tc.If(c2 > 0) as cmpo:
                    acc_pass(1, hbufs[0])
                    for t in range(2, n_steps):
                        ct = load_count()
                        with tc.If(ct > 0):
                            step_body(t)
                    final_copies()
                with cmpo.Else():
                    acc_pass(1, hbufs[0], to_out=True)

        # ---- final outputs -------------------------------------------
        nc.sync.dma_start(stats_e.ap()[0:1, :], steps_r[:])
        nc.sync.dma_start(stats_e.ap()[1:2, :], rem_v[:])
        nc.sync.dma_start(stats_e.ap()[2:3, :], nat_r[:])
        nc.sync.dma_start(stats_e.ap()[3:4, :], forc_r[:])
        nc.sync.dma_start(curve_e.ap()[:, :], curve_sb[:])


# --------------------------------------------------------------- runner ----
_CACHE = {}


def _get_nc(n_steps=T):
    if n_steps not in _CACHE:
        _CACHE[n_steps] = _build(n_steps)[0]
    return _CACHE[n_steps]


def _marshal(inputs):
    """Host-side input marshaling: shard x, pre-transpose / tile-block /
    bf16-cast the replicated weights into the layouts the device consumes."""
    bf = ml_dtypes.bfloat16
    x = np.asarray(inputs["input_tensor"], dtype=np.float32)
    w_ih = np.asarray(inputs["weight_ih"], dtype=np.float32)
    w_hh = np.asarray(inputs["weight_hh"], dtype=np.float32)
    b_ih = np.asarray(inputs["bias_ih"], dtype=np.float32)
    b_hh = np.asarray(inputs["bias_hh"], dtype=np.float32)
    halt_w = np.asarray(inputs["halt_w"], dtype=np.float32)
    halt_b = np.asarray(inputs["halt_b"], dtype=np.float32)

    # w_ih_t[jt, p, it*128+j'] = w_ih[jt*128+j', it*128+p]
    wih_blk = w_ih[:, :IN].reshape(GT, 128, IT, 128)  # [jt, j', it, p]
    wih_blk = np.ascontiguousarray(
        wih_blk.transpose(0, 3, 2, 1).reshape(GT, 128, IT * 128)
    ).astype(bf)

    # w_hh_t[jt, p, kt*128+j'] = w_hh[jt*128+j', kt*128+p]
    whh_blk = w_hh.reshape(GT, 128, KT, 128)  # [jt, j', kt, p]
    whh_blk = np.ascontiguousarray(
        whh_blk.transpose(0, 3, 2, 1).reshape(GT, 128, KT * 128)
    ).astype(bf)

    bih_p = np.ascontiguousarray(b_ih.reshape(GT, 128).T)
    bhh_p = np.ascontiguousarray(b_hh.reshape(GT, 128).T)
    wf_p = np.ascontiguousarray(w_ih[:, IN].reshape(GT, 128).T)
    hw_p = np.ascontiguousarray(halt_w.reshape(KT, 128).T.astype(bf))
    hb_p = np.ascontiguousarray(halt_b.reshape(1, 1))

    base = {
        "w_ih_t": wih_blk, "w_hh_t": whh_blk,
        "bih_p": bih_p, "bhh_p": bhh_p, "wf_p": wf_p,
        "hw_p": hw_p, "hb_p": hb_p,
    }
    in_maps = []
    for ci in range(N_CORES):
        xs = x[ci * B:(ci + 1) * B]  # [B, IN]
        # x_t[p, it*B + b] = x[b, it*128+p]
        xt = np.ascontiguousarray(
            xs.reshape(B, IT, 128).transpose(2, 1, 0).reshape(128, IT * B)
        ).astype(bf)
        in_maps.append(dict(base, x_t=xt))
    return in_maps


def run_device(inputs, n_steps=T, trace=False):
    from concourse.bass_utils import run_bass_kernel_spmd

    nc = _get_nc(n_steps)
    in_maps = _marshal(inputs)
    return run_bass_kernel_spmd(nc, in_maps, core_ids=list(range(N_CORES)),
                                trace=trace)


def combine(results):
    """Host-side unshard + tiny final statistics."""
    accs, steps_l, rem_l, curves, tmaxes = [], [], [], [], []
    nat_s = forc_s = 0.0
    for ci in range(N_CORES):
        r = results[ci]
        accs.append(np.ascontiguousarray(r["acc_t"].T))
        stats = r["stats"]
        steps_l.append(stats[0])
        rem_l.append(stats[1])
        nat_s += float(stats[2].sum(dtype=np.float64))
        forc_s += float(stats[3].sum(dtype=np.float64))
        curves.append(r["curve"][0, :T].copy())
        tmaxes.append(int(stats[0].max()))

    acc_state = np.concatenate(accs, axis=0)
    steps = np.concatenate(steps_l)
    rem_v = np.concatenate(rem_l)

    # pad each core's curve past its last executed step with its final value
    curve = np.zeros(T, dtype=np.float64)
    for ci in range(N_CORES):
        c = curves[ci].astype(np.float64)
        tm = max(tmaxes[ci], 1)
        c[tm:] = c[tm - 1]
        curve += c
    curve = (curve / B_FULL).astype(np.float32)

    ponder = (steps + rem_v) * TIME_PENALTY
    final_ponder = np.float32(ponder.mean(dtype=np.float64))
    remainder_mean = np.float32(rem_v.mean(dtype=np.float64))
    remainder_std = np.float32(rem_v.std(dtype=np.float64))
    natural_ratio = np.float32(nat_s / B_FULL)
    forced_ratio = np.float32(forc_s / B_FULL)
    p50 = np.float32(np.quantile(steps.astype(np.float64), 0.5))
    p90 = np.float32(np.quantile(steps.astype(np.float64), 0.9))
    return (acc_state, final_ponder, steps, remainder_mean, remainder_std,
            natural_ratio, forced_ratio, p50, p90, curve)


def kernel(**inputs):
    res = run_device(inputs, n_steps=T, trace=False)
    return combine(res.results)


if __name__ == "__main__":
    import os
    import time

    t0 = time.time()
    nc, nfix = _build(int(os.environ.get("NSTEPS", T)))
    print(f"built ok in {time.time() - t0:.1f}s, waitsplit fixes: {nfix}")
